# revision 27
# baseline (speedup 1.0000x reference)
"""DeepSeek-V3.2 MLA attention on 8 Trainium2 NeuronCores (Bass/Tile).

Strategy (tensor parallel over heads, per the sharding hint):
  Launch A: sequence-sharded latent projections, token-major. Core c
    computes q/kv down-projections + RMSNorm for its 256-token slice
    with 512-wide moving operands (weights moving, hidden stationary),
    ssq fused into tensor_tensor_reduce on the DVE, and the normalize
    applied straight out of PSUM by the ACT engine (per-partition
    scale), so there is no copy tail. Host transposes to feature-major.
  Launch B: head-sharded attention. Core c owns heads (2c, 2c+1).
    For a block-causal mask (verified on host at 128x128 granularity),
    the kernel skips fully-masked key tiles, restricts the diagonal
    tiles' matmuls to their unmasked query columns, applies one shared
    128x128 staircase mask pattern on the DVE, computes the softmax
    denominator with per-tile ones-matmuls on the PE (no serial DVE
    chain), and interleaves up-projection / attention / deferred
    o-projection so the PE stream stays dense (HAM stays warm).
    Host sums the 8 partial outputs (the all-reduce after o_proj).

Host-side precomputation folds gqa/gkva into Wqb/Wkvb rows and the
softmax 1/sqrt(192) into the q-latent normalization (layout/dtype prep
only - all FLOPs of the module run on device).
"""

import numpy as np

import concourse.bass as bass
import concourse.tile as tile
from concourse import bacc, mybir
from concourse.bass_utils import run_bass_kernel_spmd

F32 = mybir.dt.float32
F32R = mybir.dt.float32r
BF16 = mybir.dt.bfloat16

S = 2048
HID = 2048
QL = 1536
KVL = 512
ROPE = 64
NOPE = 128
VH = 128
NH = 16
NCORES = 8
HPC = NH // NCORES          # heads per core = 2
SL = S // NCORES            # token slice per core in launch A = 256
QLT = QL // 128             # 12
KVT = KVL // 128            # 4
HT = HID // 128             # 16
ST = S // 128               # 16
EPS = 1e-6
QFC = 3                     # q feature chunks of 512 in launch A

_CACHE = {}


def _build_a():
    """Launch A: latents for a 256-token slice, token-major, bf16.

    in : hsl [128, HT*SL]  hidden slice, hid-major (partition=hid%128)
         wq  [128, QFC*HT*512]  Wqa, fc-major then j-major
         wkv [128, HT*KVL]      Wkva latent part, j-major
         wrp [128, HT*ROPE]     Wkva rope part, j-major
    out: qtok  [SL, QL]   rmsnorm(hidden@Wqa)/sqrt(192)  (g folded later)
         kvtok [SL, KVL]  rmsnorm-normalized kv latent
         rptok [SL, ROPE] raw shared k_rope
    """
    nc = bacc.Bacc("TRN2", target_bir_lowering=False, debug=False,
                   num_devices=NCORES)
    hsl = nc.dram_tensor("hsl", [128, HT * SL], BF16,
                         kind="ExternalInput").ap()
    wq = nc.dram_tensor("wq", [128, QFC * HT * 512], BF16,
                        kind="ExternalInput").ap()
    wkv = nc.dram_tensor("wkv", [128, HT * KVL], BF16,
                         kind="ExternalInput").ap()
    wrp = nc.dram_tensor("wrp", [128, HT * ROPE], BF16,
                         kind="ExternalInput").ap()
    qtok = nc.dram_tensor("qtok", [SL, QL], BF16, kind="ExternalOutput").ap()
    kvr = nc.dram_tensor("kvr", [SL, KVL + ROPE], BF16,
                         kind="ExternalOutput").ap()

    TT = SL // 128  # 2 token tiles

    with tile.TileContext(nc) as tc:
        with tc.tile_pool(name="w", bufs=1) as wp, \
             tc.tile_pool(name="sc", bufs=2) as scp, \
             tc.tile_pool(name="st", bufs=24) as stp, \
             tc.tile_pool(name="out", bufs=2) as outp, \
             tc.tile_pool(name="ps", bufs=7, space="PSUM") as pq:
            # partition-sliced loads: each DMA covers all columns of a
            # 16-partition band, so DRAM rows are 8-16KB (vs 1KB for
            # column slices) and descriptor-issue cost stops dominating.
            def pload(dst, src, cols, nsl=8):
                step = 128 // nsl
                for p in range(nsl):
                    nc.sync.dma_start(dst[p * step:(p + 1) * step, :cols],
                                      src[p * step:(p + 1) * step, :cols])

            ht = wp.tile([128, HT * SL], BF16, tag="ht")
            pload(ht, hsl, HT * SL)
            ht_t = [ht[:, j * SL:(j + 1) * SL] for j in range(HT)]
            wq_t = {}
            for fc in range(QFC):
                t = wp.tile([128, HT * 512], BF16, tag=f"wq{fc}",
                            name=f"wq{fc}")
                pload(t, wq[:, fc * HT * 512:(fc + 1) * HT * 512], HT * 512)
                for j in range(HT):
                    wq_t[fc, j] = t[:, j * 512:(j + 1) * 512]
            wkvt = wp.tile([128, HT * KVL], BF16, tag="wkv")
            pload(wkvt, wkv, HT * KVL)
            wkv_t = [wkvt[:, j * KVL:(j + 1) * KVL] for j in range(HT)]
            wrp_s = wp.tile([128, HT * ROPE], BF16, tag="wrp")
            pload(wrp_s, wrp, HT * ROPE, nsl=2)

            epsq = wp.tile([128, 1], F32, tag="epsq")
            nc.vector.memset(epsq[:], 192.0 * EPS)
            epsk = wp.tile([128, 1], F32, tag="epsk")
            nc.vector.memset(epsk[:], EPS)

            def chain(tt, mov_of, width):
                """16-deep contraction chain into one PSUM tile."""
                ps = pq.tile([128, 512], F32, tag="ps")
                for j in range(HT):
                    nc.tensor.matmul(
                        ps[:, :width],
                        ht_t[j][:, tt * 128:tt * 128 + 128],
                        mov_of(j),
                        start=(j == 0), stop=(j == HT - 1))
                return ps

            # ---- q path: 3 feature chunks x 2 token tiles ----
            q_ps = [[None] * TT for _ in range(QFC)]
            q_ssq = [None] * TT
            for fc in range(QFC):
                for tt in range(TT):
                    ps = chain(tt, lambda j: wq_t[fc, j][:], 512)
                    q_ps[fc][tt] = ps
                    sc = scp.tile([128, 512], F32, tag="sc")
                    acc = stp.tile([128, 1], F32, tag="st")
                    nc.scalar.activation(
                        sc[:], ps[:], mybir.ActivationFunctionType.Square,
                        accum_out=acc[:])
                    if fc == 0:
                        q_ssq[tt] = acc
                    else:
                        nacc = stp.tile([128, 1], F32, tag="st")
                        nc.vector.tensor_add(nacc[:], q_ssq[tt][:], acc[:])
                        q_ssq[tt] = nacc
                    if fc == QFC - 1:
                        acc = q_ssq[tt]
                        # rr = 1/sqrt(ssq*(192/QL) + 192*eps): folds the
                        # softmax 1/sqrt(192) into the rmsnorm scale.
                        sd = stp.tile([128, 1], F32, tag="st")
                        nc.scalar.activation(
                            sd[:], acc[:], mybir.ActivationFunctionType.Sqrt,
                            bias=epsq[:], scale=192.0 / QL)
                        rr = stp.tile([128, 1], F32, tag="st")
                        nc.vector.reciprocal_approx_fast(rr[:], sd[:])
                        o = outp.tile([128, QL], BF16, tag="qo")
                        for f2 in range(QFC):
                            nc.scalar.mul(o[:, f2 * 512:(f2 + 1) * 512],
                                          q_ps[f2][tt][:], rr[:])
                        for p in range(2):  # full 3KB rows, 2 queues
                            nc.sync.dma_start(
                                qtok[tt * 128 + p * 64:tt * 128 + (p + 1) * 64,
                                     :],
                                o[p * 64:(p + 1) * 64, :])

            # ---- kv + rope path (combined output rows) ----
            for tt in range(TT):
                ps = chain(tt, lambda j: wkv_t[j][:], 512)
                sc = scp.tile([128, 512], F32, tag="sc")
                acc = stp.tile([128, 1], F32, tag="st")
                nc.scalar.activation(
                    sc[:], ps[:], mybir.ActivationFunctionType.Square,
                    accum_out=acc[:])
                sd = stp.tile([128, 1], F32, tag="st")
                nc.scalar.activation(
                    sd[:], acc[:], mybir.ActivationFunctionType.Sqrt,
                    bias=epsk[:], scale=1.0 / KVL)
                rr = stp.tile([128, 1], F32, tag="st")
                nc.vector.reciprocal_approx_fast(rr[:], sd[:])
                o = outp.tile([128, KVL + ROPE], BF16, tag="ko")
                nc.scalar.mul(o[:, :KVL], ps[:], rr[:])
                psr = chain(tt, lambda j: wrp_s[:, j * ROPE:(j + 1) * ROPE],
                            ROPE)
                nc.scalar.copy(o[:, KVL:], psr[:, :ROPE])
                for p in range(2):
                    nc.sync.dma_start(
                        kvr[tt * 128 + p * 64:tt * 128 + (p + 1) * 64, :],
                        o[p * 64:(p + 1) * 64, :])
    nc.compile()
    return nc


def _build_b_causal():
    """Launch B (block-causal mask): 2 heads of attention + o-proj partial.

    in : qlat [QL, S], kvlat [KVL, S], rp [ROPE, S]  (feature-major latents)
         mstair [128, 128] (the shared diagonal staircase mask, [k, q]),
         wqn [128, QLT*HPC*NOPE], wqr [128, QLT*HPC*64],
         wkn [128, KVT*HPC*NOPE], wkv [128, KVT*HPC*VH], wo [HPC*128, HID]
    out: part [S, HID] bf16 (this core's 2-head contribution)
    """
    nc = bacc.Bacc("TRN2", target_bir_lowering=False, debug=False,
                   num_devices=NCORES)
    qlat = nc.dram_tensor("qlat", [QL, S], BF16, kind="ExternalInput").ap()
    kvlat = nc.dram_tensor("kvlat", [KVL, S], BF16,
                           kind="ExternalInput").ap()
    rp = nc.dram_tensor("rp", [ROPE, S], BF16, kind="ExternalInput").ap()
    mstair = nc.dram_tensor("mstair", [128, 128], BF16,
                            kind="ExternalInput").ap()
    wqn = nc.dram_tensor("wqn", [128, QLT * HPC * NOPE], BF16,
                         kind="ExternalInput").ap()
    wqr = nc.dram_tensor("wqr", [128, QLT * HPC * 64], BF16,
                         kind="ExternalInput").ap()
    wkn = nc.dram_tensor("wkn", [128, KVT * HPC * NOPE], BF16,
                         kind="ExternalInput").ap()
    wkv = nc.dram_tensor("wkv", [128, KVT * HPC * VH], BF16,
                         kind="ExternalInput").ap()
    wo = nc.dram_tensor("wo", [HPC * VH, HID], BF16,
                        kind="ExternalInput").ap()
    part = nc.dram_tensor("part", [S, HID], BF16, kind="ExternalOutput").ap()

    CH = 512            # up-projection chunk == attention query chunk
    NCH = S // CH       # 4
    QC = CH

    with tile.TileContext(nc) as tc:
        with tc.tile_pool(name="w", bufs=1) as wp, \
             tc.tile_pool(name="act", bufs=1) as ap_, \
             tc.tile_pool(name="lq", bufs=1) as lqp, \
             tc.tile_pool(name="tmp", bufs=2) as tp, \
             tc.tile_pool(name="et", bufs=4) as ep, \
             tc.tile_pool(name="ot", bufs=2) as otp, \
             tc.tile_pool(name="fo", bufs=3) as fop, \
             tc.tile_pool(name="es", bufs=2) as esp, \
             tc.tile_pool(name="ps", bufs=3, space="PSUM") as pp, \
             tc.tile_pool(name="psden", bufs=1, space="PSUM") as pdp, \
             tc.tile_pool(name="pspv", bufs=2, space="PSUM") as pvp, \
             tc.tile_pool(name="pso", bufs=2, space="PSUM") as pop:
            ones_f = wp.tile([128, 1], F32, tag="ones")
            nc.vector.memset(ones_f[:], 1.0)
            ones = ones_f[:].bitcast(F32R)
            zb = wp.tile([128, 1], F32, tag="zb")
            nc.vector.memset(zb[:], 0.0)

            # ---- persistent per-head activations (feature-major) ----
            qn_T = [ap_.tile([128, S], BF16, tag=f"qnT{h}", name=f"qnT{h}")
                    for h in range(HPC)]
            qr2_T = ap_.tile([128, S], BF16, tag="qr2T")
            kn_T = [ap_.tile([128, S], BF16, tag=f"knT{h}", name=f"knT{h}")
                    for h in range(HPC)]
            v2 = ap_.tile([128, ST * HPC * VH], BF16, tag="v2")
            kr2_T = ap_.tile([128, S], BF16, tag="kr2T")

            def psliced(dst, src, cols, nsl):
                step = 128 // nsl
                for p in range(nsl):
                    nc.sync.dma_start(dst[p * step:(p + 1) * step, :cols],
                                      src[p * step:(p + 1) * step, :cols])

            def load_pair(c):
                """Load latent chunks c and c+1 with 2KB-row column slices."""
                w2 = 2 * CH
                csl = slice(c * CH, (c + 2) * CH)
                lq = lqp.tile([128, QLT * w2], BF16, tag="lqP", name="lqP")
                for m in range(QLT):
                    nc.sync.dma_start(lq[:, m * w2:(m + 1) * w2],
                                      qlat[m * 128:(m + 1) * 128, csl])
                lk = lqp.tile([128, KVT * w2], BF16, tag="lkP", name="lkP")
                for m in range(KVT):
                    nc.sync.dma_start(lk[:, m * w2:(m + 1) * w2],
                                      kvlat[m * 128:(m + 1) * 128, csl])
                nc.sync.dma_start(kr2_T[0:64, csl], rp[:, csl])
                nc.sync.dma_start(kr2_T[64:128, csl], rp[:, csl])
                out = []
                for d in range(2):
                    lqs = [lq[:, m * w2 + d * CH:m * w2 + (d + 1) * CH]
                           for m in range(QLT)]
                    lks = [lk[:, m * w2 + d * CH:m * w2 + (d + 1) * CH]
                           for m in range(KVT)]
                    out.append((lqs, lks))
                return out

            def load_chunk(c):
                csl = slice(c * CH, (c + 1) * CH)
                lq = lqp.tile([128, QLT * CH], BF16, tag="lq", name="lq")
                for m in range(QLT):
                    nc.sync.dma_start(lq[:, m * CH:(m + 1) * CH],
                                      qlat[m * 128:(m + 1) * 128, csl])
                lk = lqp.tile([128, KVT * CH], BF16, tag="lk", name="lk")
                for m in range(KVT):
                    nc.sync.dma_start(lk[:, m * CH:(m + 1) * CH],
                                      kvlat[m * 128:(m + 1) * 128, csl])
                nc.sync.dma_start(kr2_T[0:64, csl], rp[:, csl])
                nc.sync.dma_start(kr2_T[64:128, csl], rp[:, csl])
                lqs = [lq[:, m * CH:(m + 1) * CH] for m in range(QLT)]
                lks = [lk[:, m * CH:(m + 1) * CH] for m in range(KVT)]
                return lqs, lks

            # ---- preamble. kv-path inputs first: up_proj starts with the
            # kn/v chains, so the PE can begin while the q latents stream.
            # Chunk-0 latents use per-m tiles (dep tracking is
            # tile-granular) so each chain only waits for its own slice.
            # Chunks 1+2 load as a 2KB-row pair; wo streams last. ----
            lk0 = []
            for m in range(KVT):
                t = lqp.tile([128, CH], BF16, tag=f"lk0_{m}",
                             name=f"lk0_{m}")
                nc.sync.dma_start(t[:], kvlat[m * 128:(m + 1) * 128, 0:CH])
                lk0.append(t[:])
            wkn_s = wp.tile([128, KVT * HPC * NOPE], BF16, tag="wkn")
            psliced(wkn_s, wkn, KVT * HPC * NOPE, 4)
            wkv_s = wp.tile([128, KVT * HPC * VH], BF16, tag="wkv")
            psliced(wkv_s, wkv, KVT * HPC * VH, 4)
            lq0 = []
            for m in range(QLT):
                t = lqp.tile([128, CH], BF16, tag=f"lq0_{m}",
                             name=f"lq0_{m}")
                nc.sync.dma_start(t[:], qlat[m * 128:(m + 1) * 128, 0:CH])
                lq0.append(t[:])
            wqn_s = wp.tile([128, QLT * HPC * NOPE], BF16, tag="wqn")
            psliced(wqn_s, wqn, QLT * HPC * NOPE, 8)
            nc.sync.dma_start(kr2_T[0:64, 0:CH], rp[:, 0:CH])
            nc.sync.dma_start(kr2_T[64:128, 0:CH], rp[:, 0:CH])
            wqr_s = wp.tile([128, QLT * HPC * 64], BF16, tag="wqr")
            psliced(wqr_s, wqr, QLT * HPC * 64, 4)
            md_s = wp.tile([128, 128], BF16, tag="mstair")
            nc.sync.dma_start(md_s[:], mstair[:, :])
            pend = [(lq0, lk0)] + load_pair(1)
            wo_s = wp.tile([128, HPC * HID], BF16, tag="wo")
            for h in range(HPC):
                psliced(wo_s[:, h * HID:(h + 1) * HID],
                        wo[h * 128:(h + 1) * 128, :], HID, 4)

            def up_proj(c, lq, lk):
                csl = slice(c * CH, (c + 1) * CH)
                # kv-path chains first: they only need the (small) k latents
                for h in range(HPC):
                    ps = pp.tile([128, CH], F32, tag="ups")
                    for m in range(KVT):
                        nc.tensor.matmul(
                            ps[:],
                            wkn_s[:, m * HPC * NOPE + h * NOPE:
                                  m * HPC * NOPE + (h + 1) * NOPE],
                            lk[m],
                            start=(m == 0), stop=(m == KVT - 1))
                    nc.scalar.copy(kn_T[h][:, csl], ps[:])
                for st in range(CH // 128):
                    ps = pp.tile([128, CH], F32, tag="ups")
                    for m in range(KVT):
                        nc.tensor.matmul(
                            ps[:, :HPC * VH],
                            lk[m][:, st * 128:(st + 1) * 128],
                            wkv_s[:, m * HPC * VH:(m + 1) * HPC * VH],
                            start=(m == 0), stop=(m == KVT - 1))
                    gst = c * (CH // 128) + st
                    nc.scalar.copy(
                        v2[:, gst * HPC * VH:(gst + 1) * HPC * VH],
                        ps[:, :HPC * VH])
                for h in range(HPC):
                    ps = pp.tile([128, CH], F32, tag="ups")
                    for m in range(QLT):
                        nc.tensor.matmul(
                            ps[:],
                            wqn_s[:, m * HPC * NOPE + h * NOPE:
                                  m * HPC * NOPE + (h + 1) * NOPE],
                            lq[m],
                            start=(m == 0), stop=(m == QLT - 1))
                    nc.vector.tensor_copy(qn_T[h][:, csl], ps[:])
                ps = pp.tile([128, CH], F32, tag="ups")
                for m in range(QLT):
                    nc.tensor.matmul(ps[:],
                                     wqr_s[:, m * HPC * 64:(m + 1) * HPC * 64],
                                     lq[m],
                                     start=(m == 0), stop=(m == QLT - 1))
                nc.vector.tensor_copy(qr2_T[:, csl], ps[:])

            def attention(qc):
                """Causal attention for query chunk qc; returns ot tiles.

                Full key tiles first, then the 4 diagonal tiles restricted
                to their unmasked query columns. The exp sums accumulate in
                SBUF on GpSimd (even tiles) and DVE (odd tiles); one f32r
                ones-matmul per head turns the sum into the denominator.
                """
                qb = qc * QC
                tiles = [(kt, 0) for kt in range(4 * qc)]
                tiles += [(4 * qc + d, 128 * d) for d in range(4)]
                n = len(tiles)
                ot = []
                for h in range(HPC):
                    ps_o = pvp.tile([128, QC], F32, tag="po")
                    es = []
                    for p in range(2):
                        t = esp.tile([128, QC], F32, tag=f"es{p}")
                        (nc.gpsimd if p == 0 else nc.vector).memset(t[:], 0.0)
                        es.append(t)
                    prev = None

                    def pv(i, kt, off, et):
                        w = QC - off
                        nc.tensor.matmul(
                            ps_o[:, off:],
                            v2[:, kt * HPC * VH + h * VH:
                               kt * HPC * VH + (h + 1) * VH],
                            et[:, :w], start=(i == 0), stop=(i == n - 1))

                    for i, (kt, off) in enumerate(tiles):
                        w = QC - off
                        ps_s = pp.tile([128, QC], F32, tag="ups")
                        nc.tensor.matmul(ps_s[:, :w],
                                         kn_T[h][:, kt * 128:(kt + 1) * 128],
                                         qn_T[h][:, qb + off:qb + QC],
                                         start=True, stop=False)
                        nc.tensor.matmul(
                            ps_s[:, :w],
                            kr2_T[h * 64:(h + 1) * 64,
                                  kt * 128:(kt + 1) * 128],
                            qr2_T[h * 64:(h + 1) * 64, qb + off:qb + QC],
                            start=False, stop=True)
                        if kt >= 4 * qc:    # diagonal tile: staircase mask
                            nc.vector.tensor_add(
                                ps_s[:, :128], ps_s[:, :128], md_s[:])
                        et = ep.tile([128, QC], BF16, tag="et")
                        nc.scalar.activation(
                            et[:, :w], ps_s[:, :w],
                            mybir.ActivationFunctionType.Exp,
                            bias=zb[:], scale=1.0)
                        eng = nc.gpsimd if i % 2 == 0 else nc.vector
                        e = es[i % 2]
                        eng.tensor_add(e[:, off:], e[:, off:], et[:, :w])
                        if prev is not None:
                            pv(*prev)
                        prev = (i, kt, off, et)
                    pv(*prev)
                    est = tp.tile([128, QC], F32R, tag="est")
                    nc.vector.tensor_add(est[:], es[0][:], es[1][:])
                    ps_den = pdp.tile([1, QC], F32, tag="den")
                    nc.tensor.matmul(ps_den[:], ones, est[:],
                                     start=True, stop=True)
                    rd = tp.tile([1, QC], F32, tag="rd")
                    dencp = tp.tile([1, QC], F32, tag="dencp")
                    nc.vector.tensor_copy(dencp[:], ps_den[:])
                    nc.vector.reciprocal_approx_fast(rd[:], dencp[:])
                    rdb = tp.tile([128, QC], F32, tag="rdb")
                    nc.gpsimd.partition_broadcast(rdb[:], rd[:1])
                    o = otp.tile([128, QC], BF16, tag=f"ot{h}")
                    nc.vector.tensor_mul(o[:], ps_o[:], rdb[:])
                    ot.append(o)
                return ot

            def o_proj(qc, ot):
                for st in range(QC // 128):
                    foc = fop.tile([128, HID], BF16, tag="fo")
                    for nn in range(HID // 512):
                        ps_f = pop.tile([128, 512], F32, tag="pf")
                        for h in range(HPC):
                            nc.tensor.matmul(
                                ps_f[:],
                                ot[h][:, st * 128:(st + 1) * 128],
                                wo_s[:, h * HID + nn * 512:
                                     h * HID + (nn + 1) * 512],
                                start=(h == 0), stop=(h == HPC - 1))
                        if nn % 2 == 0:
                            nc.vector.tensor_copy(
                                foc[:, nn * 512:(nn + 1) * 512], ps_f[:])
                        else:
                            nc.scalar.copy(
                                foc[:, nn * 512:(nn + 1) * 512], ps_f[:])
                    for p in range(2):  # full 4KB rows, 2 queues
                        nc.sync.dma_start(
                            part[qc * QC + st * 128 + p * 64:
                                 qc * QC + st * 128 + (p + 1) * 64, :],
                            foc[p * 64:(p + 1) * 64, :])

            chunks = pend
            prev_ot = None
            for c in range(NCH):
                if c == 1:
                    chunks.append(load_chunk(3))
                lq, lk = chunks[c]
                up_proj(c, lq, lk)
                if prev_ot is not None:
                    o_proj(c - 1, prev_ot)
                prev_ot = attention(c)
            o_proj(NCH - 1, prev_ot)
    nc.compile()
    return nc


def _build_b_general():
    """Fallback launch B for arbitrary masks: full [S,S] mask, no tile
    skipping (bf16 activations)."""
    nc = bacc.Bacc("TRN2", target_bir_lowering=False, debug=False,
                   num_devices=NCORES)
    qlat = nc.dram_tensor("qlat", [QL, S], BF16, kind="ExternalInput").ap()
    kvlat = nc.dram_tensor("kvlat", [KVL, S], BF16,
                           kind="ExternalInput").ap()
    rp = nc.dram_tensor("rp", [ROPE, S], BF16, kind="ExternalInput").ap()
    maskT = nc.dram_tensor("maskT", [S, S], BF16,
                           kind="ExternalInput").ap()
    wqn = nc.dram_tensor("wqn", [128, QLT * HPC * NOPE], BF16,
                         kind="ExternalInput").ap()
    wqr = nc.dram_tensor("wqr", [128, QLT * HPC * 64], BF16,
                         kind="ExternalInput").ap()
    wkn = nc.dram_tensor("wkn", [128, KVT * HPC * NOPE], BF16,
                         kind="ExternalInput").ap()
    wkv = nc.dram_tensor("wkv", [128, KVT * HPC * VH], BF16,
                         kind="ExternalInput").ap()
    wo = nc.dram_tensor("wo", [HPC * VH, HID], BF16,
                        kind="ExternalInput").ap()
    part = nc.dram_tensor("part", [S, HID], BF16, kind="ExternalOutput").ap()

    CH = 512
    NCH = S // CH
    QC = CH

    with tile.TileContext(nc) as tc:
        with tc.tile_pool(name="w", bufs=1) as wp, \
             tc.tile_pool(name="act", bufs=1) as ap_, \
             tc.tile_pool(name="lq", bufs=2) as lqp, \
             tc.tile_pool(name="msk", bufs=24) as mp, \
             tc.tile_pool(name="tmp", bufs=2) as tp, \
             tc.tile_pool(name="et", bufs=3) as ep, \
             tc.tile_pool(name="out", bufs=5) as op, \
             tc.tile_pool(name="ps", bufs=2, space="PSUM") as pp, \
             tc.tile_pool(name="psden", bufs=2, space="PSUM") as pdp, \
             tc.tile_pool(name="pspv", bufs=2, space="PSUM") as pvp, \
             tc.tile_pool(name="pso", bufs=2, space="PSUM") as pop:
            ones_b = wp.tile([128, 1], BF16, tag="ones")
            nc.vector.memset(ones_b[:], 1.0)
            ones = ones_b[:]
            zb = wp.tile([128, 1], F32, tag="zb")
            nc.vector.memset(zb[:], 0.0)

            qn_T = [ap_.tile([128, S], BF16, tag=f"qnT{h}", name=f"qnT{h}")
                    for h in range(HPC)]
            qr2_T = ap_.tile([128, S], BF16, tag="qr2T")
            kn_T = [ap_.tile([128, S], BF16, tag=f"knT{h}", name=f"knT{h}")
                    for h in range(HPC)]
            v2 = ap_.tile([128, ST * HPC * VH], BF16, tag="v2")
            kr2_T = ap_.tile([128, S], BF16, tag="kr2T")

            def load_chunk(c):
                csl = slice(c * CH, (c + 1) * CH)
                lq = lqp.tile([128, QLT * CH], BF16, tag="lq", name="lq")
                for m in range(QLT):
                    nc.sync.dma_start(lq[:, m * CH:(m + 1) * CH],
                                      qlat[m * 128:(m + 1) * 128, csl])
                lk = lqp.tile([128, KVT * CH], BF16, tag="lk", name="lk")
                for m in range(KVT):
                    nc.sync.dma_start(lk[:, m * CH:(m + 1) * CH],
                                      kvlat[m * 128:(m + 1) * 128, csl])
                nc.sync.dma_start(kr2_T[0:64, csl], rp[:, csl])
                nc.sync.dma_start(kr2_T[64:128, csl], rp[:, csl])
                return lq, lk

            pend = load_chunk(0)
            wqn_s = wp.tile([128, QLT * HPC * NOPE], BF16, tag="wqn")
            for m in range(QLT):
                nc.sync.dma_start(
                    wqn_s[:, m * HPC * NOPE:(m + 1) * HPC * NOPE],
                    wqn[:, m * HPC * NOPE:(m + 1) * HPC * NOPE])
            wqr_s = wp.tile([128, QLT * HPC * 64], BF16, tag="wqr")
            nc.sync.dma_start(wqr_s[:], wqr[:, :])
            wkn_s = wp.tile([128, KVT * HPC * NOPE], BF16, tag="wkn")
            nc.sync.dma_start(wkn_s[:], wkn[:, :])
            wkv_s = wp.tile([128, KVT * HPC * VH], BF16, tag="wkv")
            nc.sync.dma_start(wkv_s[:], wkv[:, :])
            wo_s = wp.tile([128, HPC * HID], BF16, tag="wo")
            for h in range(HPC):
                nc.sync.dma_start(wo_s[:, h * HID:(h + 1) * HID],
                                  wo[h * 128:(h + 1) * 128, :])

            def up_proj(c, lq, lk):
                csl = slice(c * CH, (c + 1) * CH)
                for h in range(HPC):
                    ps = pp.tile([128, CH], F32, tag="ups")
                    for m in range(QLT):
                        nc.tensor.matmul(
                            ps[:],
                            wqn_s[:, m * HPC * NOPE + h * NOPE:
                                  m * HPC * NOPE + (h + 1) * NOPE],
                            lq[:, m * CH:(m + 1) * CH],
                            start=(m == 0), stop=(m == QLT - 1))
                    nc.vector.tensor_copy(qn_T[h][:, csl], ps[:])
                ps = pp.tile([128, CH], F32, tag="ups")
                for m in range(QLT):
                    nc.tensor.matmul(ps[:],
                                     wqr_s[:, m * HPC * 64:(m + 1) * HPC * 64],
                                     lq[:, m * CH:(m + 1) * CH],
                                     start=(m == 0), stop=(m == QLT - 1))
                nc.vector.tensor_copy(qr2_T[:, csl], ps[:])
                for h in range(HPC):
                    ps = pp.tile([128, CH], F32, tag="ups")
                    for m in range(KVT):
                        nc.tensor.matmul(
                            ps[:],
                            wkn_s[:, m * HPC * NOPE + h * NOPE:
                                  m * HPC * NOPE + (h + 1) * NOPE],
                            lk[:, m * CH:(m + 1) * CH],
                            start=(m == 0), stop=(m == KVT - 1))
                    nc.scalar.copy(kn_T[h][:, csl], ps[:])
                for st in range(CH // 128):
                    ps = pp.tile([128, HPC * VH], F32, tag="ups")
                    for m in range(KVT):
                        nc.tensor.matmul(
                            ps[:],
                            lk[:, m * CH + st * 128:m * CH + (st + 1) * 128],
                            wkv_s[:, m * HPC * VH:(m + 1) * HPC * VH],
                            start=(m == 0), stop=(m == KVT - 1))
                    gst = c * (CH // 128) + st
                    nc.scalar.copy(
                        v2[:, gst * HPC * VH:(gst + 1) * HPC * VH], ps[:])

            for c in range(NCH):
                lq, lk = pend
                if c + 1 < NCH:
                    pend = load_chunk(c + 1)
                up_proj(c, lq, lk)

            def attention(qc):
                qsl = slice(qc * QC, (qc + 1) * QC)
                mts = []
                for kt in range(ST):
                    mt = mp.tile([128, QC], BF16, tag="mask")
                    nc.sync.dma_start(mt[:],
                                      maskT[kt * 128:(kt + 1) * 128, qsl])
                    mts.append(mt)
                ot = []
                for h in range(HPC):
                    ps_den = pdp.tile([1, QC], F32, tag="den")
                    ps_o = pvp.tile([128, QC], F32, tag="po")
                    ets = {}
                    for kt in range(ST):
                        ps_s = pp.tile([128, QC], F32, tag="ups")
                        nc.tensor.matmul(ps_s[:],
                                         kn_T[h][:, kt * 128:(kt + 1) * 128],
                                         qn_T[h][:, qsl],
                                         start=True, stop=False)
                        nc.tensor.matmul(
                            ps_s[:],
                            kr2_T[h * 64:(h + 1) * 64,
                                  kt * 128:(kt + 1) * 128],
                            qr2_T[h * 64:(h + 1) * 64, qsl],
                            start=False, stop=True)
                        nc.vector.tensor_add(ps_s[:], ps_s[:], mts[kt][:])
                        et = ep.tile([128, QC], BF16, tag="et")
                        nc.scalar.activation(
                            et[:], ps_s[:], mybir.ActivationFunctionType.Exp,
                            bias=zb[:], scale=1.0)
                        ets[kt] = et
                        if kt > 0:
                            pkt = kt - 1
                            pet = ets.pop(pkt)
                            nc.tensor.matmul(
                                ps_o[:],
                                v2[:, pkt * HPC * VH + h * VH:
                                   pkt * HPC * VH + (h + 1) * VH],
                                pet[:], start=(pkt == 0), stop=False)
                            nc.tensor.matmul(ps_den[:], ones, pet[:],
                                             start=(pkt == 0), stop=False)
                    pkt = ST - 1
                    pet = ets.pop(pkt)
                    nc.tensor.matmul(
                        ps_o[:],
                        v2[:, pkt * HPC * VH + h * VH:
                           pkt * HPC * VH + (h + 1) * VH],
                        pet[:], start=(pkt == 0), stop=True)
                    nc.tensor.matmul(ps_den[:], ones, pet[:],
                                     start=(pkt == 0), stop=True)
                    rd = tp.tile([1, QC], F32, tag="rd")
                    dencp = tp.tile([1, QC], F32, tag="dencp")
                    nc.vector.tensor_copy(dencp[:], ps_den[:])
                    nc.vector.reciprocal_approx_fast(rd[:], dencp[:])
                    rdb = tp.tile([128, QC], F32, tag="rdb")
                    nc.gpsimd.partition_broadcast(rdb[:], rd[:1])
                    o = op.tile([128, QC], BF16, tag=f"ot{h}")
                    nc.vector.tensor_mul(o[:], ps_o[:], rdb[:])
                    ot.append(o)
                return ot

            def o_proj(qc, ot):
                for st in range(QC // 128):
                    for nn in range(HID // 512):
                        ps_f = pop.tile([128, 512], F32, tag="pf")
                        for h in range(HPC):
                            nc.tensor.matmul(
                                ps_f[:],
                                ot[h][:, st * 128:(st + 1) * 128],
                                wo_s[:, h * HID + nn * 512:
                                     h * HID + (nn + 1) * 512],
                                start=(h == 0), stop=(h == HPC - 1))
                        fo = op.tile([128, 512], BF16, tag="fo")
                        nc.scalar.copy(fo[:], ps_f[:])
                        nc.sync.dma_start(
                            part[qc * QC + st * 128:qc * QC + (st + 1) * 128,
                                 nn * 512:(nn + 1) * 512], fo[:])

            prev_ot = None
            for qc in range(NCH):
                if prev_ot is not None:
                    o_proj(qc - 1, prev_ot)
                prev_ot = attention(qc)
            o_proj(NCH - 1, prev_ot)
    nc.compile()
    return nc


def _check_causal128(maskT):
    """True iff maskT ([k, q], f32) is block-causal at 128x128 tile
    granularity with one shared diagonal pattern; returns (ok, P[128,128])."""
    P = None
    for qt in range(ST):
        for kt in range(ST):
            blk = maskT[kt * 128:(kt + 1) * 128, qt * 128:(qt + 1) * 128]
            if kt < qt:
                if not np.all(blk == 0.0):
                    return False, None
            elif kt > qt:
                if not np.all(blk <= -1e8):
                    return False, None
            elif P is None:
                P = blk
            elif not np.array_equal(P, blk):
                return False, None
    return True, P


def _get(name):
    if name not in _CACHE:
        _CACHE[name] = {"a": _build_a, "bc": _build_b_causal,
                        "bg": _build_b_general}[name]()
    return _CACHE[name]


def _prep(hidden_states, attention_mask, Wqa, gqa, Wqb, Wkva, gkva, Wkvb, Wo):
    import ml_dtypes
    f = np.float32
    bf = ml_dtypes.bfloat16
    hid_T = np.ascontiguousarray(hidden_states[0].T).astype(bf)
    mask_T = np.ascontiguousarray(
        np.asarray(attention_mask[0, 0], f).T)
    ok, mstair = _check_causal128(mask_T)
    Wqb_g = (np.asarray(gqa, f)[:, None] * np.asarray(Wqb, f)).astype(f)
    Wkvb_g = (np.asarray(gkva, f)[:, None] * np.asarray(Wkvb, f)).astype(f)
    # launch-A weight layouts: hid-partition-major, j(-contraction)-sliced
    wqa_np = np.asarray(Wqa, f)
    wkva_np = np.asarray(Wkva, f)
    wq_b = np.ascontiguousarray(
        wqa_np.reshape(HT, 128, QFC, 512).transpose(1, 2, 0, 3)
        .reshape(128, QFC * HT * 512)).astype(bf)
    wkv_b = np.ascontiguousarray(
        wkva_np[:, :KVL].reshape(HT, 128, KVL).transpose(1, 0, 2)
        .reshape(128, HT * KVL)).astype(bf)
    wrp_b = np.ascontiguousarray(
        wkva_np[:, KVL:].reshape(HT, 128, ROPE).transpose(1, 0, 2)
        .reshape(128, HT * ROPE)).astype(bf)
    ins_a, ins_b = [], []
    for c in range(NCORES):
        hsl_c = np.ascontiguousarray(
            hid_T[:, c * SL:(c + 1) * SL].reshape(HT, 128, SL)
            .transpose(1, 0, 2).reshape(128, HT * SL))
        ins_a.append({
            "hsl": hsl_c,
            "wq": wq_b,
            "wkv": wkv_b,
            "wrp": wrp_b,
        })
        heads = [HPC * c + h for h in range(HPC)]
        wqn = np.concatenate([Wqb_g[:, h * 192:h * 192 + NOPE] for h in heads],
                             axis=1)
        wqr = np.concatenate([Wqb_g[:, h * 192 + NOPE:(h + 1) * 192]
                              for h in heads], axis=1)
        wkn = np.concatenate([Wkvb_g[:, h * 256:h * 256 + NOPE]
                              for h in heads], axis=1)
        wkv = np.concatenate([Wkvb_g[:, h * 256 + NOPE:(h + 1) * 256]
                              for h in heads], axis=1)
        wo = np.concatenate([np.asarray(Wo, f)[h * VH:(h + 1) * VH, :]
                             for h in heads], axis=0)
        mask_in = ({"mstair": mstair.astype(bf)} if ok
                   else {"maskT": mask_T.astype(bf)})

        def perm(w, nt):
            # [nt*128, F] -> [128, nt*F] tile-major contiguous
            return np.ascontiguousarray(
                w.reshape(nt, 128, w.shape[1]).transpose(1, 0, 2)
                .reshape(128, nt * w.shape[1])).astype(bf)

        ins_b.append({
            **mask_in,
            "wqn": perm(wqn, QLT),
            "wqr": perm(wqr, QLT),
            "wkn": perm(wkn, KVT),
            "wkv": perm(wkv, KVT),
            "wo": np.ascontiguousarray(wo).astype(bf),
        })
    return ins_a, ins_b, ("bc" if ok else "bg")


def _run(ins_a, ins_b, bname="bc", trace=False):
    core_ids = list(range(NCORES))
    res_a = run_bass_kernel_spmd(_get("a"), ins_a, core_ids, trace=trace)
    qlat = np.ascontiguousarray(np.concatenate(
        [res_a.results[c]["qtok"] for c in range(NCORES)], axis=0).T)
    kvr = np.concatenate([res_a.results[c]["kvr"] for c in range(NCORES)],
                         axis=0)
    kvlat = np.ascontiguousarray(kvr[:, :KVL].T)
    rplat = np.ascontiguousarray(kvr[:, KVL:].T)
    for m in ins_b:
        m["qlat"] = qlat
        m["kvlat"] = kvlat
        m["rp"] = rplat
    res_b = run_bass_kernel_spmd(_get(bname), ins_b, core_ids, trace=trace)
    out = res_b.results[0]["part"].astype(np.float32)
    for c in range(1, NCORES):
        out = out + res_b.results[c]["part"].astype(np.float32)
    return out[None], res_a, res_b


def kernel(hidden_states, attention_mask, Wqa, gqa, Wqb, Wkva, gkva, Wkvb, Wo):
    ins_a, ins_b, bname = _prep(hidden_states, attention_mask, Wqa, gqa, Wqb,
                                Wkva, gkva, Wkvb, Wo)
    out, _, _ = _run(ins_a, ins_b, bname)
    return out


# revision 34
# speedup vs baseline: 1.1812x; 1.1812x over previous
"""DeepSeek-V3.2 MLA attention on 8 Trainium2 NeuronCores (Bass/Tile).

Strategy (tensor parallel over heads, per the sharding hint):
  Launch A: sequence-sharded latent projections, token-major. Core c
    computes q/kv down-projections + RMSNorm for its 256-token slice
    with 512-wide moving operands (weights moving, hidden stationary),
    ssq fused into tensor_tensor_reduce on the DVE, and the normalize
    applied straight out of PSUM by the ACT engine (per-partition
    scale), so there is no copy tail. Host transposes to feature-major.
  Launch B: head-sharded attention. Core c owns heads (2c, 2c+1).
    For a block-causal mask (verified on host at 128x128 granularity),
    the kernel skips fully-masked key tiles, restricts the diagonal
    tiles' matmuls to their unmasked query columns, applies one shared
    128x128 staircase mask pattern on the DVE, computes the softmax
    denominator with per-tile ones-matmuls on the PE (no serial DVE
    chain), and interleaves up-projection / attention / deferred
    o-projection so the PE stream stays dense (HAM stays warm).
    Host sums the 8 partial outputs (the all-reduce after o_proj).

Host-side precomputation folds gqa/gkva into Wqb/Wkvb rows and the
softmax 1/sqrt(192) into the q-latent normalization (layout/dtype prep
only - all FLOPs of the module run on device).
"""

import numpy as np

import concourse.bass as bass
import concourse.tile as tile
from concourse import bacc, mybir
from concourse.bass_utils import run_bass_kernel_spmd

F32 = mybir.dt.float32
F32R = mybir.dt.float32r
BF16 = mybir.dt.bfloat16

S = 2048
HID = 2048
QL = 1536
KVL = 512
ROPE = 64
NOPE = 128
VH = 128
NH = 16
NCORES = 8
HPC = NH // NCORES          # heads per core = 2
SL = S // NCORES            # token slice per core in launch A = 256
QLT = QL // 128             # 12
KVT = KVL // 128            # 4
HT = HID // 128             # 16
ST = S // 128               # 16
EPS = 1e-6
QFC = 3                     # q feature chunks of 512 in launch A

_CACHE = {}


def _build_a():
    """Launch A: latents for a 256-token slice, token-major, bf16.

    in : hsl [128, HT*SL]  hidden slice, hid-major (partition=hid%128)
         wq  [128, QFC*HT*512]  Wqa, fc-major then j-major
         wkv [128, HT*KVL]      Wkva latent part, j-major
         wrp [128, HT*ROPE]     Wkva rope part, j-major
    out: qtok  [SL, QL]   rmsnorm(hidden@Wqa)/sqrt(192)  (g folded later)
         kvtok [SL, KVL]  rmsnorm-normalized kv latent
         rptok [SL, ROPE] raw shared k_rope
    """
    nc = bacc.Bacc("TRN2", target_bir_lowering=False, debug=False,
                   num_devices=NCORES)
    hsl = nc.dram_tensor("hsl", [128, HT * SL], BF16,
                         kind="ExternalInput").ap()
    wq = nc.dram_tensor("wq", [128, QFC * HT * 512], BF16,
                        kind="ExternalInput").ap()
    wkv = nc.dram_tensor("wkv", [128, HT * KVL], BF16,
                         kind="ExternalInput").ap()
    wrp = nc.dram_tensor("wrp", [128, HT * ROPE], BF16,
                         kind="ExternalInput").ap()
    qtok = nc.dram_tensor("qtok", [SL, QL], BF16, kind="ExternalOutput").ap()
    kvr = nc.dram_tensor("kvr", [SL, KVL + ROPE], BF16,
                         kind="ExternalOutput").ap()

    TT = SL // 128  # 2 token tiles

    with tile.TileContext(nc) as tc:
        with tc.tile_pool(name="w", bufs=1) as wp, \
             tc.tile_pool(name="sc", bufs=2) as scp, \
             tc.tile_pool(name="st", bufs=24) as stp, \
             tc.tile_pool(name="out", bufs=2) as outp, \
             tc.tile_pool(name="ps", bufs=7, space="PSUM") as pq:
            # DMA pieces sized for the queue model: ~64-128KB per dma with
            # >=2KB DRAM rows ([pstep, 1024]-col slices); many pieces
            # round-robin across the 16 queues.
            def sload(dst, src, cols, piece=1024, pstep=64):
                for c0 in range(0, cols, piece):
                    w = min(piece, cols - c0)
                    for p0 in range(0, 128, pstep):
                        nc.sync.dma_start(dst[p0:p0 + pstep, c0:c0 + w],
                                          src[p0:p0 + pstep, c0:c0 + w])

            ht = wp.tile([128, HT * SL], BF16, tag="ht")
            sload(ht, hsl, HT * SL, pstep=32)
            ht_t = [ht[:, j * SL:(j + 1) * SL] for j in range(HT)]
            wq_t = {}
            for fc in range(QFC):
                t = wp.tile([128, HT * 512], BF16, tag=f"wq{fc}",
                            name=f"wq{fc}")
                sload(t, wq[:, fc * HT * 512:(fc + 1) * HT * 512], HT * 512)
                for j in range(HT):
                    wq_t[fc, j] = t[:, j * 512:(j + 1) * 512]
            wkvt = wp.tile([128, HT * KVL], BF16, tag="wkv")
            sload(wkvt, wkv, HT * KVL)
            wkv_t = [wkvt[:, j * KVL:(j + 1) * KVL] for j in range(HT)]
            wrp_s = wp.tile([128, HT * ROPE], BF16, tag="wrp")
            sload(wrp_s, wrp, HT * ROPE)

            epsq = wp.tile([128, 1], F32, tag="epsq")
            nc.vector.memset(epsq[:], 192.0 * EPS)
            epsk = wp.tile([128, 1], F32, tag="epsk")
            nc.vector.memset(epsk[:], EPS)

            def chain(tt, mov_of, width):
                """16-deep contraction chain into one PSUM tile."""
                ps = pq.tile([128, 512], F32, tag="ps")
                for j in range(HT):
                    nc.tensor.matmul(
                        ps[:, :width],
                        ht_t[j][:, tt * 128:tt * 128 + 128],
                        mov_of(j),
                        start=(j == 0), stop=(j == HT - 1))
                return ps

            # ---- q path: 3 feature chunks x 2 token tiles ----
            q_ps = [[None] * TT for _ in range(QFC)]
            q_ssq = [None] * TT
            for fc in range(QFC):
                for tt in range(TT):
                    ps = chain(tt, lambda j: wq_t[fc, j][:], 512)
                    q_ps[fc][tt] = ps
                    sc = scp.tile([128, 512], F32, tag="sc")
                    acc = stp.tile([128, 1], F32, tag="st")
                    nc.scalar.activation(
                        sc[:], ps[:], mybir.ActivationFunctionType.Square,
                        accum_out=acc[:])
                    if fc == 0:
                        q_ssq[tt] = acc
                    else:
                        nacc = stp.tile([128, 1], F32, tag="st")
                        nc.vector.tensor_add(nacc[:], q_ssq[tt][:], acc[:])
                        q_ssq[tt] = nacc
                    if fc == QFC - 1:
                        acc = q_ssq[tt]
                        # rr = 1/sqrt(ssq*(192/QL) + 192*eps): folds the
                        # softmax 1/sqrt(192) into the rmsnorm scale.
                        sd = stp.tile([128, 1], F32, tag="st")
                        nc.scalar.activation(
                            sd[:], acc[:], mybir.ActivationFunctionType.Sqrt,
                            bias=epsq[:], scale=192.0 / QL)
                        rr = stp.tile([128, 1], F32, tag="st")
                        nc.vector.reciprocal_approx_fast(rr[:], sd[:])
                        o = outp.tile([128, QL], BF16, tag="qo")
                        for f2 in range(QFC):
                            nc.scalar.mul(o[:, f2 * 512:(f2 + 1) * 512],
                                          q_ps[f2][tt][:], rr[:])
                        for p in range(4):  # full 3KB rows, 4 queues
                            nc.sync.dma_start(
                                qtok[tt * 128 + p * 32:tt * 128 + (p + 1) * 32,
                                     :],
                                o[p * 32:(p + 1) * 32, :])

            # ---- kv + rope path (combined output rows) ----
            for tt in range(TT):
                ps = chain(tt, lambda j: wkv_t[j][:], 512)
                sc = scp.tile([128, 512], F32, tag="sc")
                acc = stp.tile([128, 1], F32, tag="st")
                nc.scalar.activation(
                    sc[:], ps[:], mybir.ActivationFunctionType.Square,
                    accum_out=acc[:])
                sd = stp.tile([128, 1], F32, tag="st")
                nc.scalar.activation(
                    sd[:], acc[:], mybir.ActivationFunctionType.Sqrt,
                    bias=epsk[:], scale=1.0 / KVL)
                rr = stp.tile([128, 1], F32, tag="st")
                nc.vector.reciprocal_approx_fast(rr[:], sd[:])
                o = outp.tile([128, KVL + ROPE], BF16, tag="ko")
                nc.scalar.mul(o[:, :KVL], ps[:], rr[:])
                psr = chain(tt, lambda j: wrp_s[:, j * ROPE:(j + 1) * ROPE],
                            ROPE)
                nc.scalar.copy(o[:, KVL:], psr[:, :ROPE])
                for p in range(4):
                    nc.sync.dma_start(
                        kvr[tt * 128 + p * 32:tt * 128 + (p + 1) * 32, :],
                        o[p * 32:(p + 1) * 32, :])
    nc.compile()
    return nc


def _build_b_causal():
    """Launch B (block-causal mask): 2 heads of attention + o-proj partial.

    in : qlat [QL, S], kvlat [KVL, S], rp [ROPE, S]  (feature-major latents)
         mstair [128, 128] (the shared diagonal staircase mask, [k, q]),
         wqn [128, QLT*HPC*NOPE], wqr [128, QLT*HPC*64],
         wkn [128, KVT*HPC*NOPE], wkv [128, KVT*HPC*VH], wo [HPC*128, HID]
    out: part [S, HID] bf16 (this core's 2-head contribution)
    """
    nc = bacc.Bacc("TRN2", target_bir_lowering=False, debug=False,
                   num_devices=NCORES)
    qlat = nc.dram_tensor("qlat", [QL, S], BF16, kind="ExternalInput").ap()
    kvlat = nc.dram_tensor("kvlat", [KVL, S], BF16,
                           kind="ExternalInput").ap()
    rp = nc.dram_tensor("rp", [ROPE, S], BF16, kind="ExternalInput").ap()
    mstair = nc.dram_tensor("mstair", [128, 128], BF16,
                            kind="ExternalInput").ap()
    wqn = nc.dram_tensor("wqn", [128, QLT * HPC * NOPE], BF16,
                         kind="ExternalInput").ap()
    wqr = nc.dram_tensor("wqr", [128, QLT * HPC * 64], BF16,
                         kind="ExternalInput").ap()
    wkn = nc.dram_tensor("wkn", [128, KVT * HPC * NOPE], BF16,
                         kind="ExternalInput").ap()
    wkv = nc.dram_tensor("wkv", [128, KVT * HPC * VH], BF16,
                         kind="ExternalInput").ap()
    wo = nc.dram_tensor("wo", [HPC * VH, HID], BF16,
                        kind="ExternalInput").ap()
    part = nc.dram_tensor("part", [S, HID], BF16, kind="ExternalOutput").ap()

    CH = 512            # up-projection chunk == attention query chunk
    NCH = S // CH       # 4
    QC = CH

    with tile.TileContext(nc) as tc:
        with tc.tile_pool(name="w", bufs=1) as wp, \
             tc.tile_pool(name="act", bufs=1) as ap_, \
             tc.tile_pool(name="lq", bufs=1) as lqp, \
             tc.tile_pool(name="tmp", bufs=2) as tp, \
             tc.tile_pool(name="et", bufs=4) as ep, \
             tc.tile_pool(name="ot", bufs=2) as otp, \
             tc.tile_pool(name="fo", bufs=3) as fop, \
             tc.tile_pool(name="es", bufs=2) as esp, \
             tc.tile_pool(name="ps", bufs=3, space="PSUM") as pp, \
             tc.tile_pool(name="psden", bufs=1, space="PSUM") as pdp, \
             tc.tile_pool(name="pspv", bufs=2, space="PSUM") as pvp, \
             tc.tile_pool(name="pso", bufs=2, space="PSUM") as pop:
            ones_f = wp.tile([128, 1], F32, tag="ones")
            nc.vector.memset(ones_f[:], 1.0)
            ones = ones_f[:].bitcast(F32R)
            zb = wp.tile([128, 1], F32, tag="zb")
            nc.vector.memset(zb[:], 0.0)

            # ---- persistent per-head activations (feature-major) ----
            qn_T = [ap_.tile([128, S], BF16, tag=f"qnT{h}", name=f"qnT{h}")
                    for h in range(HPC)]
            qr2_T = ap_.tile([128, S], BF16, tag="qr2T")
            kn_T = [ap_.tile([128, S], BF16, tag=f"knT{h}", name=f"knT{h}")
                    for h in range(HPC)]
            v2 = ap_.tile([128, ST * HPC * VH], BF16, tag="v2")
            kr2_T = ap_.tile([128, S], BF16, tag="kr2T")

            def sload(dst, src, cols, piece=1024, pstep=64):
                for c0 in range(0, cols, piece):
                    w = min(piece, cols - c0)
                    for p0 in range(0, 128, pstep):
                        nc.sync.dma_start(dst[p0:p0 + pstep, c0:c0 + w],
                                          src[p0:p0 + pstep, c0:c0 + w])

            def load_pair(c):
                """Load latent chunks c and c+1 with 2KB-row column slices."""
                w2 = 2 * CH
                csl = slice(c * CH, (c + 2) * CH)
                lq = lqp.tile([128, QLT * w2], BF16, tag="lqP", name="lqP")
                for m in range(QLT):
                    sload(lq[:, m * w2:(m + 1) * w2],
                          qlat[m * 128:(m + 1) * 128, csl], w2)
                lk = lqp.tile([128, KVT * w2], BF16, tag="lkP", name="lkP")
                for m in range(KVT):
                    sload(lk[:, m * w2:(m + 1) * w2],
                          kvlat[m * 128:(m + 1) * 128, csl], w2)
                nc.sync.dma_start(kr2_T[0:64, csl], rp[:, csl])
                nc.sync.dma_start(kr2_T[64:128, csl], rp[:, csl])
                out = []
                for d in range(2):
                    lqs = [lq[:, m * w2 + d * CH:m * w2 + (d + 1) * CH]
                           for m in range(QLT)]
                    lks = [lk[:, m * w2 + d * CH:m * w2 + (d + 1) * CH]
                           for m in range(KVT)]
                    out.append((lqs, lks))
                return out

            def load_chunk(c):
                csl = slice(c * CH, (c + 1) * CH)
                lq = lqp.tile([128, QLT * CH], BF16, tag="lq", name="lq")
                for m in range(QLT):
                    sload(lq[:, m * CH:(m + 1) * CH],
                          qlat[m * 128:(m + 1) * 128, csl], CH)
                lk = lqp.tile([128, KVT * CH], BF16, tag="lk", name="lk")
                for m in range(KVT):
                    sload(lk[:, m * CH:(m + 1) * CH],
                          kvlat[m * 128:(m + 1) * 128, csl], CH)
                nc.sync.dma_start(kr2_T[0:64, csl], rp[:, csl])
                nc.sync.dma_start(kr2_T[64:128, csl], rp[:, csl])
                lqs = [lq[:, m * CH:(m + 1) * CH] for m in range(QLT)]
                lks = [lk[:, m * CH:(m + 1) * CH] for m in range(KVT)]
                return lqs, lks

            # ---- preamble. kv-path inputs first: up_proj starts with the
            # kn/v chains, so the PE can begin while the q latents stream.
            # Chunk-0 latents use per-m tiles (dep tracking is
            # tile-granular) so each chain only waits for its own slice.
            # Chunks 1+2 load as a 2KB-row pair; wo streams last. ----
            lk0 = []
            for m in range(KVT):
                t = lqp.tile([128, CH], BF16, tag=f"lk0_{m}",
                             name=f"lk0_{m}")
                sload(t, kvlat[m * 128:(m + 1) * 128, 0:CH], CH, pstep=32)
                lk0.append(t[:])
            wkn_s = wp.tile([128, KVT * HPC * NOPE], BF16, tag="wkn")
            sload(wkn_s, wkn, KVT * HPC * NOPE, pstep=32)
            wkv_s = wp.tile([128, KVT * HPC * VH], BF16, tag="wkv")
            sload(wkv_s, wkv, KVT * HPC * VH, pstep=32)
            lq0 = []
            for m in range(QLT):
                t = lqp.tile([128, CH], BF16, tag=f"lq0_{m}",
                             name=f"lq0_{m}")
                sload(t, qlat[m * 128:(m + 1) * 128, 0:CH], CH)
                lq0.append(t[:])
            wqn_s = wp.tile([128, QLT * HPC * NOPE], BF16, tag="wqn")
            sload(wqn_s, wqn, QLT * HPC * NOPE, pstep=32)
            nc.sync.dma_start(kr2_T[0:64, 0:CH], rp[:, 0:CH])
            nc.sync.dma_start(kr2_T[64:128, 0:CH], rp[:, 0:CH])
            wqr_s = wp.tile([128, QLT * HPC * 64], BF16, tag="wqr")
            sload(wqr_s, wqr, QLT * HPC * 64)
            md_s = wp.tile([128, 128], BF16, tag="mstair")
            nc.sync.dma_start(md_s[:], mstair[:, :])
            pend = [(lq0, lk0)] + load_pair(1)
            wo_s = wp.tile([128, HPC * HID], BF16, tag="wo")
            for h in range(HPC):
                sload(wo_s[:, h * HID:(h + 1) * HID],
                      wo[h * 128:(h + 1) * 128, :], HID)

            def up_proj(c, lq, lk):
                csl = slice(c * CH, (c + 1) * CH)
                # kv-path chains first: they only need the (small) k latents
                for h in range(HPC):
                    ps = pp.tile([128, CH], F32, tag="ups")
                    for m in range(KVT):
                        nc.tensor.matmul(
                            ps[:],
                            wkn_s[:, m * HPC * NOPE + h * NOPE:
                                  m * HPC * NOPE + (h + 1) * NOPE],
                            lk[m],
                            start=(m == 0), stop=(m == KVT - 1))
                    nc.scalar.copy(kn_T[h][:, csl], ps[:])
                for st in range(CH // 128):
                    ps = pp.tile([128, CH], F32, tag="ups")
                    for m in range(KVT):
                        nc.tensor.matmul(
                            ps[:, :HPC * VH],
                            lk[m][:, st * 128:(st + 1) * 128],
                            wkv_s[:, m * HPC * VH:(m + 1) * HPC * VH],
                            start=(m == 0), stop=(m == KVT - 1))
                    gst = c * (CH // 128) + st
                    nc.scalar.copy(
                        v2[:, gst * HPC * VH:(gst + 1) * HPC * VH],
                        ps[:, :HPC * VH])
                for h in range(HPC):
                    ps = pp.tile([128, CH], F32, tag="ups")
                    for m in range(QLT):
                        nc.tensor.matmul(
                            ps[:],
                            wqn_s[:, m * HPC * NOPE + h * NOPE:
                                  m * HPC * NOPE + (h + 1) * NOPE],
                            lq[m],
                            start=(m == 0), stop=(m == QLT - 1))
                    nc.vector.tensor_copy(qn_T[h][:, csl], ps[:])
                ps = pp.tile([128, CH], F32, tag="ups")
                for m in range(QLT):
                    nc.tensor.matmul(ps[:],
                                     wqr_s[:, m * HPC * 64:(m + 1) * HPC * 64],
                                     lq[m],
                                     start=(m == 0), stop=(m == QLT - 1))
                nc.vector.tensor_copy(qr2_T[:, csl], ps[:])

            def attention(qc):
                """Causal attention for query chunk qc; returns ot tiles.

                Full key tiles first, then the 4 diagonal tiles restricted
                to their unmasked query columns. The exp sums accumulate in
                SBUF on GpSimd (even tiles) and DVE (odd tiles); one f32r
                ones-matmul per head turns the sum into the denominator.
                """
                qb = qc * QC
                tiles = [(kt, 0) for kt in range(4 * qc)]
                tiles += [(4 * qc + d, 128 * d) for d in range(4)]
                n = len(tiles)
                ot = []
                for h in range(HPC):
                    ps_o = pvp.tile([128, QC], F32, tag="po")
                    es = []
                    for p in range(2):
                        t = esp.tile([128, QC], F32, tag=f"es{p}")
                        (nc.gpsimd if p == 0 else nc.vector).memset(t[:], 0.0)
                        es.append(t)
                    prev = None

                    def pv(i, kt, off, et):
                        w = QC - off
                        nc.tensor.matmul(
                            ps_o[:, off:],
                            v2[:, kt * HPC * VH + h * VH:
                               kt * HPC * VH + (h + 1) * VH],
                            et[:, :w], start=(i == 0), stop=(i == n - 1))

                    for i, (kt, off) in enumerate(tiles):
                        w = QC - off
                        ps_s = pp.tile([128, QC], F32, tag="ups")
                        nc.tensor.matmul(ps_s[:, :w],
                                         kn_T[h][:, kt * 128:(kt + 1) * 128],
                                         qn_T[h][:, qb + off:qb + QC],
                                         start=True, stop=False)
                        nc.tensor.matmul(
                            ps_s[:, :w],
                            kr2_T[h * 64:(h + 1) * 64,
                                  kt * 128:(kt + 1) * 128],
                            qr2_T[h * 64:(h + 1) * 64, qb + off:qb + QC],
                            start=False, stop=True)
                        if kt >= 4 * qc:    # diagonal tile: staircase mask
                            nc.vector.tensor_add(
                                ps_s[:, :128], ps_s[:, :128], md_s[:])
                        et = ep.tile([128, QC], BF16, tag="et")
                        nc.scalar.activation(
                            et[:, :w], ps_s[:, :w],
                            mybir.ActivationFunctionType.Exp,
                            bias=zb[:], scale=1.0)
                        eng = nc.gpsimd if i % 2 == 0 else nc.vector
                        e = es[i % 2]
                        eng.tensor_add(e[:, off:], e[:, off:], et[:, :w])
                        if prev is not None:
                            pv(*prev)
                        prev = (i, kt, off, et)
                    pv(*prev)
                    est = tp.tile([128, QC], F32R, tag="est")
                    nc.vector.tensor_add(est[:], es[0][:], es[1][:])
                    ps_den = pdp.tile([1, QC], F32, tag="den")
                    nc.tensor.matmul(ps_den[:], ones, est[:],
                                     start=True, stop=True)
                    rd = tp.tile([1, QC], F32, tag="rd")
                    dencp = tp.tile([1, QC], F32, tag="dencp")
                    nc.vector.tensor_copy(dencp[:], ps_den[:])
                    nc.vector.reciprocal_approx_fast(rd[:], dencp[:])
                    rdb = tp.tile([128, QC], F32, tag="rdb")
                    nc.gpsimd.partition_broadcast(rdb[:], rd[:1])
                    o = otp.tile([128, QC], BF16, tag=f"ot{h}")
                    nc.vector.tensor_mul(o[:], ps_o[:], rdb[:])
                    ot.append(o)
                return ot

            def o_proj(qc, ot):
                for st in range(QC // 128):
                    foc = fop.tile([128, HID], BF16, tag="fo")
                    for nn in range(HID // 512):
                        ps_f = pop.tile([128, 512], F32, tag="pf")
                        for h in range(HPC):
                            nc.tensor.matmul(
                                ps_f[:],
                                ot[h][:, st * 128:(st + 1) * 128],
                                wo_s[:, h * HID + nn * 512:
                                     h * HID + (nn + 1) * 512],
                                start=(h == 0), stop=(h == HPC - 1))
                        if nn % 2 == 0:
                            nc.vector.tensor_copy(
                                foc[:, nn * 512:(nn + 1) * 512], ps_f[:])
                        else:
                            nc.scalar.copy(
                                foc[:, nn * 512:(nn + 1) * 512], ps_f[:])
                    for p in range(4):  # full 4KB rows, 4 queues
                        nc.sync.dma_start(
                            part[qc * QC + st * 128 + p * 32:
                                 qc * QC + st * 128 + (p + 1) * 32, :],
                            foc[p * 32:(p + 1) * 32, :])

            chunks = pend
            prev_ot = None
            for c in range(NCH):
                if c == 1:
                    chunks.append(load_chunk(3))
                lq, lk = chunks[c]
                up_proj(c, lq, lk)
                if prev_ot is not None:
                    o_proj(c - 1, prev_ot)
                prev_ot = attention(c)
            o_proj(NCH - 1, prev_ot)
    nc.compile()
    return nc


def _build_b_general():
    """Fallback launch B for arbitrary masks: full [S,S] mask, no tile
    skipping (bf16 activations)."""
    nc = bacc.Bacc("TRN2", target_bir_lowering=False, debug=False,
                   num_devices=NCORES)
    qlat = nc.dram_tensor("qlat", [QL, S], BF16, kind="ExternalInput").ap()
    kvlat = nc.dram_tensor("kvlat", [KVL, S], BF16,
                           kind="ExternalInput").ap()
    rp = nc.dram_tensor("rp", [ROPE, S], BF16, kind="ExternalInput").ap()
    maskT = nc.dram_tensor("maskT", [S, S], BF16,
                           kind="ExternalInput").ap()
    wqn = nc.dram_tensor("wqn", [128, QLT * HPC * NOPE], BF16,
                         kind="ExternalInput").ap()
    wqr = nc.dram_tensor("wqr", [128, QLT * HPC * 64], BF16,
                         kind="ExternalInput").ap()
    wkn = nc.dram_tensor("wkn", [128, KVT * HPC * NOPE], BF16,
                         kind="ExternalInput").ap()
    wkv = nc.dram_tensor("wkv", [128, KVT * HPC * VH], BF16,
                         kind="ExternalInput").ap()
    wo = nc.dram_tensor("wo", [HPC * VH, HID], BF16,
                        kind="ExternalInput").ap()
    part = nc.dram_tensor("part", [S, HID], BF16, kind="ExternalOutput").ap()

    CH = 512
    NCH = S // CH
    QC = CH

    with tile.TileContext(nc) as tc:
        with tc.tile_pool(name="w", bufs=1) as wp, \
             tc.tile_pool(name="act", bufs=1) as ap_, \
             tc.tile_pool(name="lq", bufs=2) as lqp, \
             tc.tile_pool(name="msk", bufs=24) as mp, \
             tc.tile_pool(name="tmp", bufs=2) as tp, \
             tc.tile_pool(name="et", bufs=3) as ep, \
             tc.tile_pool(name="out", bufs=5) as op, \
             tc.tile_pool(name="ps", bufs=2, space="PSUM") as pp, \
             tc.tile_pool(name="psden", bufs=2, space="PSUM") as pdp, \
             tc.tile_pool(name="pspv", bufs=2, space="PSUM") as pvp, \
             tc.tile_pool(name="pso", bufs=2, space="PSUM") as pop:
            ones_b = wp.tile([128, 1], BF16, tag="ones")
            nc.vector.memset(ones_b[:], 1.0)
            ones = ones_b[:]
            zb = wp.tile([128, 1], F32, tag="zb")
            nc.vector.memset(zb[:], 0.0)

            qn_T = [ap_.tile([128, S], BF16, tag=f"qnT{h}", name=f"qnT{h}")
                    for h in range(HPC)]
            qr2_T = ap_.tile([128, S], BF16, tag="qr2T")
            kn_T = [ap_.tile([128, S], BF16, tag=f"knT{h}", name=f"knT{h}")
                    for h in range(HPC)]
            v2 = ap_.tile([128, ST * HPC * VH], BF16, tag="v2")
            kr2_T = ap_.tile([128, S], BF16, tag="kr2T")

            def load_chunk(c):
                csl = slice(c * CH, (c + 1) * CH)
                lq = lqp.tile([128, QLT * CH], BF16, tag="lq", name="lq")
                for m in range(QLT):
                    nc.sync.dma_start(lq[:, m * CH:(m + 1) * CH],
                                      qlat[m * 128:(m + 1) * 128, csl])
                lk = lqp.tile([128, KVT * CH], BF16, tag="lk", name="lk")
                for m in range(KVT):
                    nc.sync.dma_start(lk[:, m * CH:(m + 1) * CH],
                                      kvlat[m * 128:(m + 1) * 128, csl])
                nc.sync.dma_start(kr2_T[0:64, csl], rp[:, csl])
                nc.sync.dma_start(kr2_T[64:128, csl], rp[:, csl])
                return lq, lk

            pend = load_chunk(0)
            wqn_s = wp.tile([128, QLT * HPC * NOPE], BF16, tag="wqn")
            for m in range(QLT):
                nc.sync.dma_start(
                    wqn_s[:, m * HPC * NOPE:(m + 1) * HPC * NOPE],
                    wqn[:, m * HPC * NOPE:(m + 1) * HPC * NOPE])
            wqr_s = wp.tile([128, QLT * HPC * 64], BF16, tag="wqr")
            nc.sync.dma_start(wqr_s[:], wqr[:, :])
            wkn_s = wp.tile([128, KVT * HPC * NOPE], BF16, tag="wkn")
            nc.sync.dma_start(wkn_s[:], wkn[:, :])
            wkv_s = wp.tile([128, KVT * HPC * VH], BF16, tag="wkv")
            nc.sync.dma_start(wkv_s[:], wkv[:, :])
            wo_s = wp.tile([128, HPC * HID], BF16, tag="wo")
            for h in range(HPC):
                nc.sync.dma_start(wo_s[:, h * HID:(h + 1) * HID],
                                  wo[h * 128:(h + 1) * 128, :])

            def up_proj(c, lq, lk):
                csl = slice(c * CH, (c + 1) * CH)
                for h in range(HPC):
                    ps = pp.tile([128, CH], F32, tag="ups")
                    for m in range(QLT):
                        nc.tensor.matmul(
                            ps[:],
                            wqn_s[:, m * HPC * NOPE + h * NOPE:
                                  m * HPC * NOPE + (h + 1) * NOPE],
                            lq[:, m * CH:(m + 1) * CH],
                            start=(m == 0), stop=(m == QLT - 1))
                    nc.vector.tensor_copy(qn_T[h][:, csl], ps[:])
                ps = pp.tile([128, CH], F32, tag="ups")
                for m in range(QLT):
                    nc.tensor.matmul(ps[:],
                                     wqr_s[:, m * HPC * 64:(m + 1) * HPC * 64],
                                     lq[:, m * CH:(m + 1) * CH],
                                     start=(m == 0), stop=(m == QLT - 1))
                nc.vector.tensor_copy(qr2_T[:, csl], ps[:])
                for h in range(HPC):
                    ps = pp.tile([128, CH], F32, tag="ups")
                    for m in range(KVT):
                        nc.tensor.matmul(
                            ps[:],
                            wkn_s[:, m * HPC * NOPE + h * NOPE:
                                  m * HPC * NOPE + (h + 1) * NOPE],
                            lk[:, m * CH:(m + 1) * CH],
                            start=(m == 0), stop=(m == KVT - 1))
                    nc.scalar.copy(kn_T[h][:, csl], ps[:])
                for st in range(CH // 128):
                    ps = pp.tile([128, HPC * VH], F32, tag="ups")
                    for m in range(KVT):
                        nc.tensor.matmul(
                            ps[:],
                            lk[:, m * CH + st * 128:m * CH + (st + 1) * 128],
                            wkv_s[:, m * HPC * VH:(m + 1) * HPC * VH],
                            start=(m == 0), stop=(m == KVT - 1))
                    gst = c * (CH // 128) + st
                    nc.scalar.copy(
                        v2[:, gst * HPC * VH:(gst + 1) * HPC * VH], ps[:])

            for c in range(NCH):
                lq, lk = pend
                if c + 1 < NCH:
                    pend = load_chunk(c + 1)
                up_proj(c, lq, lk)

            def attention(qc):
                qsl = slice(qc * QC, (qc + 1) * QC)
                mts = []
                for kt in range(ST):
                    mt = mp.tile([128, QC], BF16, tag="mask")
                    nc.sync.dma_start(mt[:],
                                      maskT[kt * 128:(kt + 1) * 128, qsl])
                    mts.append(mt)
                ot = []
                for h in range(HPC):
                    ps_den = pdp.tile([1, QC], F32, tag="den")
                    ps_o = pvp.tile([128, QC], F32, tag="po")
                    ets = {}
                    for kt in range(ST):
                        ps_s = pp.tile([128, QC], F32, tag="ups")
                        nc.tensor.matmul(ps_s[:],
                                         kn_T[h][:, kt * 128:(kt + 1) * 128],
                                         qn_T[h][:, qsl],
                                         start=True, stop=False)
                        nc.tensor.matmul(
                            ps_s[:],
                            kr2_T[h * 64:(h + 1) * 64,
                                  kt * 128:(kt + 1) * 128],
                            qr2_T[h * 64:(h + 1) * 64, qsl],
                            start=False, stop=True)
                        nc.vector.tensor_add(ps_s[:], ps_s[:], mts[kt][:])
                        et = ep.tile([128, QC], BF16, tag="et")
                        nc.scalar.activation(
                            et[:], ps_s[:], mybir.ActivationFunctionType.Exp,
                            bias=zb[:], scale=1.0)
                        ets[kt] = et
                        if kt > 0:
                            pkt = kt - 1
                            pet = ets.pop(pkt)
                            nc.tensor.matmul(
                                ps_o[:],
                                v2[:, pkt * HPC * VH + h * VH:
                                   pkt * HPC * VH + (h + 1) * VH],
                                pet[:], start=(pkt == 0), stop=False)
                            nc.tensor.matmul(ps_den[:], ones, pet[:],
                                             start=(pkt == 0), stop=False)
                    pkt = ST - 1
                    pet = ets.pop(pkt)
                    nc.tensor.matmul(
                        ps_o[:],
                        v2[:, pkt * HPC * VH + h * VH:
                           pkt * HPC * VH + (h + 1) * VH],
                        pet[:], start=(pkt == 0), stop=True)
                    nc.tensor.matmul(ps_den[:], ones, pet[:],
                                     start=(pkt == 0), stop=True)
                    rd = tp.tile([1, QC], F32, tag="rd")
                    dencp = tp.tile([1, QC], F32, tag="dencp")
                    nc.vector.tensor_copy(dencp[:], ps_den[:])
                    nc.vector.reciprocal_approx_fast(rd[:], dencp[:])
                    rdb = tp.tile([128, QC], F32, tag="rdb")
                    nc.gpsimd.partition_broadcast(rdb[:], rd[:1])
                    o = op.tile([128, QC], BF16, tag=f"ot{h}")
                    nc.vector.tensor_mul(o[:], ps_o[:], rdb[:])
                    ot.append(o)
                return ot

            def o_proj(qc, ot):
                for st in range(QC // 128):
                    for nn in range(HID // 512):
                        ps_f = pop.tile([128, 512], F32, tag="pf")
                        for h in range(HPC):
                            nc.tensor.matmul(
                                ps_f[:],
                                ot[h][:, st * 128:(st + 1) * 128],
                                wo_s[:, h * HID + nn * 512:
                                     h * HID + (nn + 1) * 512],
                                start=(h == 0), stop=(h == HPC - 1))
                        fo = op.tile([128, 512], BF16, tag="fo")
                        nc.scalar.copy(fo[:], ps_f[:])
                        nc.sync.dma_start(
                            part[qc * QC + st * 128:qc * QC + (st + 1) * 128,
                                 nn * 512:(nn + 1) * 512], fo[:])

            prev_ot = None
            for qc in range(NCH):
                if prev_ot is not None:
                    o_proj(qc - 1, prev_ot)
                prev_ot = attention(qc)
            o_proj(NCH - 1, prev_ot)
    nc.compile()
    return nc


def _check_causal128(maskT):
    """True iff maskT ([k, q], f32) is block-causal at 128x128 tile
    granularity with one shared diagonal pattern; returns (ok, P[128,128])."""
    P = None
    for qt in range(ST):
        for kt in range(ST):
            blk = maskT[kt * 128:(kt + 1) * 128, qt * 128:(qt + 1) * 128]
            if kt < qt:
                if not np.all(blk == 0.0):
                    return False, None
            elif kt > qt:
                if not np.all(blk <= -1e8):
                    return False, None
            elif P is None:
                P = blk
            elif not np.array_equal(P, blk):
                return False, None
    return True, P


def _get(name):
    if name not in _CACHE:
        _CACHE[name] = {"a": _build_a, "bc": _build_b_causal,
                        "bg": _build_b_general}[name]()
    return _CACHE[name]


def _prep(hidden_states, attention_mask, Wqa, gqa, Wqb, Wkva, gkva, Wkvb, Wo):
    import ml_dtypes
    f = np.float32
    bf = ml_dtypes.bfloat16
    hid_T = np.ascontiguousarray(hidden_states[0].T).astype(bf)
    mask_T = np.ascontiguousarray(
        np.asarray(attention_mask[0, 0], f).T)
    ok, mstair = _check_causal128(mask_T)
    Wqb_g = (np.asarray(gqa, f)[:, None] * np.asarray(Wqb, f)).astype(f)
    Wkvb_g = (np.asarray(gkva, f)[:, None] * np.asarray(Wkvb, f)).astype(f)
    # launch-A weight layouts: hid-partition-major, j(-contraction)-sliced
    wqa_np = np.asarray(Wqa, f)
    wkva_np = np.asarray(Wkva, f)
    wq_b = np.ascontiguousarray(
        wqa_np.reshape(HT, 128, QFC, 512).transpose(1, 2, 0, 3)
        .reshape(128, QFC * HT * 512)).astype(bf)
    wkv_b = np.ascontiguousarray(
        wkva_np[:, :KVL].reshape(HT, 128, KVL).transpose(1, 0, 2)
        .reshape(128, HT * KVL)).astype(bf)
    wrp_b = np.ascontiguousarray(
        wkva_np[:, KVL:].reshape(HT, 128, ROPE).transpose(1, 0, 2)
        .reshape(128, HT * ROPE)).astype(bf)
    ins_a, ins_b = [], []
    for c in range(NCORES):
        hsl_c = np.ascontiguousarray(
            hid_T[:, c * SL:(c + 1) * SL].reshape(HT, 128, SL)
            .transpose(1, 0, 2).reshape(128, HT * SL))
        ins_a.append({
            "hsl": hsl_c,
            "wq": wq_b,
            "wkv": wkv_b,
            "wrp": wrp_b,
        })
        heads = [HPC * c + h for h in range(HPC)]
        wqn = np.concatenate([Wqb_g[:, h * 192:h * 192 + NOPE] for h in heads],
                             axis=1)
        wqr = np.concatenate([Wqb_g[:, h * 192 + NOPE:(h + 1) * 192]
                              for h in heads], axis=1)
        wkn = np.concatenate([Wkvb_g[:, h * 256:h * 256 + NOPE]
                              for h in heads], axis=1)
        wkv = np.concatenate([Wkvb_g[:, h * 256 + NOPE:(h + 1) * 256]
                              for h in heads], axis=1)
        wo = np.concatenate([np.asarray(Wo, f)[h * VH:(h + 1) * VH, :]
                             for h in heads], axis=0)
        mask_in = ({"mstair": mstair.astype(bf)} if ok
                   else {"maskT": mask_T.astype(bf)})

        def perm(w, nt):
            # [nt*128, F] -> [128, nt*F] tile-major contiguous
            return np.ascontiguousarray(
                w.reshape(nt, 128, w.shape[1]).transpose(1, 0, 2)
                .reshape(128, nt * w.shape[1])).astype(bf)

        ins_b.append({
            **mask_in,
            "wqn": perm(wqn, QLT),
            "wqr": perm(wqr, QLT),
            "wkn": perm(wkn, KVT),
            "wkv": perm(wkv, KVT),
            "wo": np.ascontiguousarray(wo).astype(bf),
        })
    return ins_a, ins_b, ("bc" if ok else "bg")


def _run(ins_a, ins_b, bname="bc", trace=False):
    core_ids = list(range(NCORES))
    res_a = run_bass_kernel_spmd(_get("a"), ins_a, core_ids, trace=trace)
    qlat = np.ascontiguousarray(np.concatenate(
        [res_a.results[c]["qtok"] for c in range(NCORES)], axis=0).T)
    kvr = np.concatenate([res_a.results[c]["kvr"] for c in range(NCORES)],
                         axis=0)
    kvlat = np.ascontiguousarray(kvr[:, :KVL].T)
    rplat = np.ascontiguousarray(kvr[:, KVL:].T)
    for m in ins_b:
        m["qlat"] = qlat
        m["kvlat"] = kvlat
        m["rp"] = rplat
    res_b = run_bass_kernel_spmd(_get(bname), ins_b, core_ids, trace=trace)
    out = res_b.results[0]["part"].astype(np.float32)
    for c in range(1, NCORES):
        out = out + res_b.results[c]["part"].astype(np.float32)
    return out[None], res_a, res_b


def kernel(hidden_states, attention_mask, Wqa, gqa, Wqb, Wkva, gkva, Wkvb, Wo):
    ins_a, ins_b, bname = _prep(hidden_states, attention_mask, Wqa, gqa, Wqb,
                                Wkva, gkva, Wkvb, Wo)
    out, _, _ = _run(ins_a, ins_b, bname)
    return out


# revision 39
# speedup vs baseline: 1.2213x; 1.0339x over previous
"""DeepSeek-V3.2 MLA attention on 8 Trainium2 NeuronCores (Bass/Tile).

Strategy (tensor parallel over heads, per the sharding hint):
  Launch A: sequence-sharded latent projections, token-major. Core c
    computes q/kv down-projections + RMSNorm for its 256-token slice
    with 512-wide moving operands (weights moving, hidden stationary),
    ssq fused into tensor_tensor_reduce on the DVE, and the normalize
    applied straight out of PSUM by the ACT engine (per-partition
    scale), so there is no copy tail. Host transposes to feature-major.
  Launch B: head-sharded attention. Core c owns heads (2c, 2c+1).
    For a block-causal mask (verified on host at 128x128 granularity),
    the kernel skips fully-masked key tiles, restricts the diagonal
    tiles' matmuls to their unmasked query columns, applies one shared
    128x128 staircase mask pattern on the DVE, computes the softmax
    denominator with per-tile ones-matmuls on the PE (no serial DVE
    chain), and interleaves up-projection / attention / deferred
    o-projection so the PE stream stays dense (HAM stays warm).
    Host sums the 8 partial outputs (the all-reduce after o_proj).

Host-side precomputation folds gqa/gkva into Wqb/Wkvb rows and the
softmax 1/sqrt(192) into the q-latent normalization (layout/dtype prep
only - all FLOPs of the module run on device).
"""

import numpy as np

import concourse.bass as bass
import concourse.tile as tile
from concourse import bacc, mybir
from concourse.bass_utils import run_bass_kernel_spmd

F32 = mybir.dt.float32
F32R = mybir.dt.float32r
BF16 = mybir.dt.bfloat16

S = 2048
HID = 2048
QL = 1536
KVL = 512
ROPE = 64
NOPE = 128
VH = 128
NH = 16
NCORES = 8
HPC = NH // NCORES          # heads per core = 2
SL = S // NCORES            # token slice per core in launch A = 256
QLT = QL // 128             # 12
KVT = KVL // 128            # 4
HT = HID // 128             # 16
ST = S // 128               # 16
EPS = 1e-6
QFC = 3                     # q feature chunks of 512 in launch A

_CACHE = {}


def _build_a():
    """Launch A: latents for a 256-token slice, token-major, bf16.

    in : hsl [128, HT*SL]  hidden slice, hid-major (partition=hid%128)
         wq  [128, QFC*HT*512]  Wqa, fc-major then j-major
         wkv [128, HT*KVL]      Wkva latent part, j-major
         wrp [128, HT*ROPE]     Wkva rope part, j-major
    out: qtok  [SL, QL]   rmsnorm(hidden@Wqa)/sqrt(192)  (g folded later)
         kvtok [SL, KVL]  rmsnorm-normalized kv latent
         rptok [SL, ROPE] raw shared k_rope
    """
    nc = bacc.Bacc("TRN2", target_bir_lowering=False, debug=False,
                   num_devices=NCORES)
    hsl = nc.dram_tensor("hsl", [128, HT * SL], BF16,
                         kind="ExternalInput").ap()
    wq = nc.dram_tensor("wq", [128, QFC * HT * 512], BF16,
                        kind="ExternalInput").ap()
    wkv = nc.dram_tensor("wkv", [128, HT * KVL], BF16,
                         kind="ExternalInput").ap()
    wrp = nc.dram_tensor("wrp", [128, HT * ROPE], BF16,
                         kind="ExternalInput").ap()
    qtok = nc.dram_tensor("qtok", [SL, QL], BF16, kind="ExternalOutput").ap()
    kvr = nc.dram_tensor("kvr", [SL, KVL + ROPE], BF16,
                         kind="ExternalOutput").ap()

    TT = SL // 128  # 2 token tiles

    with tile.TileContext(nc) as tc:
        with tc.tile_pool(name="w", bufs=1) as wp, \
             tc.tile_pool(name="sc", bufs=2) as scp, \
             tc.tile_pool(name="st", bufs=24) as stp, \
             tc.tile_pool(name="out", bufs=2) as outp, \
             tc.tile_pool(name="ps", bufs=7, space="PSUM") as pq:
            # DMA pieces sized for the measured queue model (~128KB with
            # >=2KB DRAM rows), dispatch rotated over the three DMA-capable
            # engines so the ~0.7us per-dma issue cost doesn't serialize.
            _eng = [nc.sync, nc.gpsimd, nc.scalar]
            _ei = [0]

            def sload(dst, src, cols, piece=1024, pstep=64):
                for c0 in range(0, cols, piece):
                    w = min(piece, cols - c0)
                    for p0 in range(0, 128, pstep):
                        _eng[_ei[0] % 3].dma_start(
                            dst[p0:p0 + pstep, c0:c0 + w],
                            src[p0:p0 + pstep, c0:c0 + w])
                        _ei[0] += 1

            ht = wp.tile([128, HT * SL], BF16, tag="ht")
            sload(ht, hsl, HT * SL, piece=2048, pstep=32)
            ht_t = [ht[:, j * SL:(j + 1) * SL] for j in range(HT)]
            wq_t = {}
            for fc in range(QFC):
                t = wp.tile([128, HT * 512], BF16, tag=f"wq{fc}",
                            name=f"wq{fc}")
                sload(t, wq[:, fc * HT * 512:(fc + 1) * HT * 512], HT * 512)
                for j in range(HT):
                    wq_t[fc, j] = t[:, j * 512:(j + 1) * 512]
            wkvt = wp.tile([128, HT * KVL], BF16, tag="wkv")
            sload(wkvt, wkv, HT * KVL)
            wkv_t = [wkvt[:, j * KVL:(j + 1) * KVL] for j in range(HT)]
            wrp_s = wp.tile([128, HT * ROPE], BF16, tag="wrp")
            sload(wrp_s, wrp, HT * ROPE)

            epsq = wp.tile([128, 1], F32, tag="epsq")
            nc.vector.memset(epsq[:], 192.0 * EPS)
            epsk = wp.tile([128, 1], F32, tag="epsk")
            nc.vector.memset(epsk[:], EPS)

            def chain(tt, mov_of, width):
                """16-deep contraction chain into one PSUM tile."""
                ps = pq.tile([128, 512], F32, tag="ps")
                for j in range(HT):
                    nc.tensor.matmul(
                        ps[:, :width],
                        ht_t[j][:, tt * 128:tt * 128 + 128],
                        mov_of(j),
                        start=(j == 0), stop=(j == HT - 1))
                return ps

            # ---- q path: 3 feature chunks x 2 token tiles ----
            q_ps = [[None] * TT for _ in range(QFC)]
            q_ssq = [None] * TT
            for fc in range(QFC):
                for tt in range(TT):
                    ps = chain(tt, lambda j: wq_t[fc, j][:], 512)
                    q_ps[fc][tt] = ps
                    sc = scp.tile([128, 512], F32, tag="sc")
                    acc = stp.tile([128, 1], F32, tag="st")
                    nc.scalar.activation(
                        sc[:], ps[:], mybir.ActivationFunctionType.Square,
                        accum_out=acc[:])
                    if fc == 0:
                        q_ssq[tt] = acc
                    else:
                        nacc = stp.tile([128, 1], F32, tag="st")
                        nc.vector.tensor_add(nacc[:], q_ssq[tt][:], acc[:])
                        q_ssq[tt] = nacc
                    if fc == QFC - 1:
                        acc = q_ssq[tt]
                        # rr = 1/sqrt(ssq*(192/QL) + 192*eps): folds the
                        # softmax 1/sqrt(192) into the rmsnorm scale.
                        sd = stp.tile([128, 1], F32, tag="st")
                        nc.scalar.activation(
                            sd[:], acc[:], mybir.ActivationFunctionType.Sqrt,
                            bias=epsq[:], scale=192.0 / QL)
                        rr = stp.tile([128, 1], F32, tag="st")
                        nc.vector.reciprocal_approx_fast(rr[:], sd[:])
                        o = outp.tile([128, QL], BF16, tag="qo")
                        for f2 in range(QFC):
                            nc.scalar.mul(o[:, f2 * 512:(f2 + 1) * 512],
                                          q_ps[f2][tt][:], rr[:])
                        for p in range(4):  # full 3KB rows, 4 queues
                            nc.sync.dma_start(
                                qtok[tt * 128 + p * 32:tt * 128 + (p + 1) * 32,
                                     :],
                                o[p * 32:(p + 1) * 32, :])

            # ---- kv + rope path (combined output rows) ----
            for tt in range(TT):
                ps = chain(tt, lambda j: wkv_t[j][:], 512)
                sc = scp.tile([128, 512], F32, tag="sc")
                acc = stp.tile([128, 1], F32, tag="st")
                nc.scalar.activation(
                    sc[:], ps[:], mybir.ActivationFunctionType.Square,
                    accum_out=acc[:])
                sd = stp.tile([128, 1], F32, tag="st")
                nc.scalar.activation(
                    sd[:], acc[:], mybir.ActivationFunctionType.Sqrt,
                    bias=epsk[:], scale=1.0 / KVL)
                rr = stp.tile([128, 1], F32, tag="st")
                nc.vector.reciprocal_approx_fast(rr[:], sd[:])
                o = outp.tile([128, KVL + ROPE], BF16, tag="ko")
                nc.scalar.mul(o[:, :KVL], ps[:], rr[:])
                psr = chain(tt, lambda j: wrp_s[:, j * ROPE:(j + 1) * ROPE],
                            ROPE)
                nc.scalar.copy(o[:, KVL:], psr[:, :ROPE])
                for p in range(4):
                    nc.sync.dma_start(
                        kvr[tt * 128 + p * 32:tt * 128 + (p + 1) * 32, :],
                        o[p * 32:(p + 1) * 32, :])
    nc.compile()
    return nc


def _build_b_causal():
    """Launch B (block-causal mask): 2 heads of attention + o-proj partial.

    in : qlat [QL, S], kvlat [KVL, S], rp [ROPE, S]  (feature-major latents)
         mstair [128, 128] (the shared diagonal staircase mask, [k, q]),
         wqn [128, QLT*HPC*NOPE], wqr [128, QLT*HPC*64],
         wkn [128, KVT*HPC*NOPE], wkv [128, KVT*HPC*VH], wo [HPC*128, HID]
    out: part [S, HID] bf16 (this core's 2-head contribution)
    """
    nc = bacc.Bacc("TRN2", target_bir_lowering=False, debug=False,
                   num_devices=NCORES)
    qlat = nc.dram_tensor("qlat", [QL, S], BF16, kind="ExternalInput").ap()
    kvlat = nc.dram_tensor("kvlat", [KVL, S], BF16,
                           kind="ExternalInput").ap()
    rp = nc.dram_tensor("rp", [ROPE, S], BF16, kind="ExternalInput").ap()
    mstair = nc.dram_tensor("mstair", [128, 128], BF16,
                            kind="ExternalInput").ap()
    wqn = nc.dram_tensor("wqn", [128, QLT * HPC * NOPE], BF16,
                         kind="ExternalInput").ap()
    wqr = nc.dram_tensor("wqr", [128, QLT * HPC * 64], BF16,
                         kind="ExternalInput").ap()
    wkn = nc.dram_tensor("wkn", [128, KVT * HPC * NOPE], BF16,
                         kind="ExternalInput").ap()
    wkv = nc.dram_tensor("wkv", [128, KVT * HPC * VH], BF16,
                         kind="ExternalInput").ap()
    wo = nc.dram_tensor("wo", [HPC * VH, HID], BF16,
                        kind="ExternalInput").ap()
    part = nc.dram_tensor("part", [S, HID], BF16, kind="ExternalOutput").ap()

    CH = 512            # up-projection chunk == attention query chunk
    NCH = S // CH       # 4
    QC = CH

    with tile.TileContext(nc) as tc:
        with tc.tile_pool(name="w", bufs=1) as wp, \
             tc.tile_pool(name="act", bufs=1) as ap_, \
             tc.tile_pool(name="lq", bufs=1) as lqp, \
             tc.tile_pool(name="tmp", bufs=2) as tp, \
             tc.tile_pool(name="et", bufs=4) as ep, \
             tc.tile_pool(name="ot", bufs=2) as otp, \
             tc.tile_pool(name="fo", bufs=3) as fop, \
             tc.tile_pool(name="es", bufs=2) as esp, \
             tc.tile_pool(name="ps", bufs=3, space="PSUM") as pp, \
             tc.tile_pool(name="psden", bufs=1, space="PSUM") as pdp, \
             tc.tile_pool(name="pspv", bufs=2, space="PSUM") as pvp, \
             tc.tile_pool(name="pso", bufs=2, space="PSUM") as pop:
            ones_f = wp.tile([128, 1], F32, tag="ones")
            nc.vector.memset(ones_f[:], 1.0)
            ones = ones_f[:].bitcast(F32R)
            zb = wp.tile([128, 1], F32, tag="zb")
            nc.vector.memset(zb[:], 0.0)

            # ---- persistent per-head activations (feature-major) ----
            qn_T = [ap_.tile([128, S], BF16, tag=f"qnT{h}", name=f"qnT{h}")
                    for h in range(HPC)]
            qr2_T = ap_.tile([128, S], BF16, tag="qr2T")
            kn_T = [ap_.tile([128, S], BF16, tag=f"knT{h}", name=f"knT{h}")
                    for h in range(HPC)]
            v2 = ap_.tile([128, ST * HPC * VH], BF16, tag="v2")
            kr2_T = ap_.tile([128, S], BF16, tag="kr2T")

            _eng = [nc.sync, nc.gpsimd, nc.scalar]
            _ei = [0]

            def sload(dst, src, cols, piece=1024, pstep=64):
                for c0 in range(0, cols, piece):
                    w = min(piece, cols - c0)
                    for p0 in range(0, 128, pstep):
                        _eng[_ei[0] % 3].dma_start(
                            dst[p0:p0 + pstep, c0:c0 + w],
                            src[p0:p0 + pstep, c0:c0 + w])
                        _ei[0] += 1

            def load_pair(c):
                """Load latent chunks c and c+1 with 2KB-row column slices."""
                w2 = 2 * CH
                csl = slice(c * CH, (c + 2) * CH)
                lq = lqp.tile([128, QLT * w2], BF16, tag="lqP", name="lqP")
                for m in range(QLT):
                    sload(lq[:, m * w2:(m + 1) * w2],
                          qlat[m * 128:(m + 1) * 128, csl], w2)
                lk = lqp.tile([128, KVT * w2], BF16, tag="lkP", name="lkP")
                for m in range(KVT):
                    sload(lk[:, m * w2:(m + 1) * w2],
                          kvlat[m * 128:(m + 1) * 128, csl], w2)
                nc.sync.dma_start(kr2_T[0:64, csl], rp[:, csl])
                nc.sync.dma_start(kr2_T[64:128, csl], rp[:, csl])
                out = []
                for d in range(2):
                    lqs = [lq[:, m * w2 + d * CH:m * w2 + (d + 1) * CH]
                           for m in range(QLT)]
                    lks = [lk[:, m * w2 + d * CH:m * w2 + (d + 1) * CH]
                           for m in range(KVT)]
                    out.append((lqs, lks))
                return out

            def load_chunk(c):
                csl = slice(c * CH, (c + 1) * CH)
                lq = lqp.tile([128, QLT * CH], BF16, tag="lq", name="lq")
                for m in range(QLT):
                    sload(lq[:, m * CH:(m + 1) * CH],
                          qlat[m * 128:(m + 1) * 128, csl], CH)
                lk = lqp.tile([128, KVT * CH], BF16, tag="lk", name="lk")
                for m in range(KVT):
                    sload(lk[:, m * CH:(m + 1) * CH],
                          kvlat[m * 128:(m + 1) * 128, csl], CH)
                nc.sync.dma_start(kr2_T[0:64, csl], rp[:, csl])
                nc.sync.dma_start(kr2_T[64:128, csl], rp[:, csl])
                lqs = [lq[:, m * CH:(m + 1) * CH] for m in range(QLT)]
                lks = [lk[:, m * CH:(m + 1) * CH] for m in range(KVT)]
                return lqs, lks

            # ---- preamble. kv-path inputs first: up_proj starts with the
            # kn/v chains, so the PE can begin while the q latents stream.
            # Chunk-0 latents use per-m tiles (dep tracking is
            # tile-granular) so each chain only waits for its own slice.
            # Chunks 1+2 load as a 2KB-row pair; wo streams last. ----
            lk0 = []
            for m in range(KVT):
                t = lqp.tile([128, CH], BF16, tag=f"lk0_{m}",
                             name=f"lk0_{m}")
                sload(t, kvlat[m * 128:(m + 1) * 128, 0:CH], CH, piece=512)
                lk0.append(t[:])
            wkn_s = wp.tile([128, KVT * HPC * NOPE], BF16, tag="wkn")
            sload(wkn_s, wkn, KVT * HPC * NOPE, piece=512)
            wkv_s = wp.tile([128, KVT * HPC * VH], BF16, tag="wkv")
            sload(wkv_s, wkv, KVT * HPC * VH, piece=512)
            lq0 = []
            for m in range(QLT):
                t = lqp.tile([128, CH], BF16, tag=f"lq0_{m}",
                             name=f"lq0_{m}")
                sload(t, qlat[m * 128:(m + 1) * 128, 0:CH], CH, piece=512)
                lq0.append(t[:])
            wqn_s = wp.tile([128, QLT * HPC * NOPE], BF16, tag="wqn")
            sload(wqn_s, wqn, QLT * HPC * NOPE)
            nc.sync.dma_start(kr2_T[0:64, 0:CH], rp[:, 0:CH])
            nc.sync.dma_start(kr2_T[64:128, 0:CH], rp[:, 0:CH])
            wqr_s = wp.tile([128, QLT * HPC * 64], BF16, tag="wqr")
            sload(wqr_s, wqr, QLT * HPC * 64)
            md_s = wp.tile([128, 128], BF16, tag="mstair")
            nc.sync.dma_start(md_s[:], mstair[:, :])
            pend = [(lq0, lk0)] + load_pair(1)
            wo_s = wp.tile([128, HPC * HID], BF16, tag="wo")
            for h in range(HPC):
                sload(wo_s[:, h * HID:(h + 1) * HID],
                      wo[h * 128:(h + 1) * 128, :], HID)

            def up_proj(c, lq, lk):
                csl = slice(c * CH, (c + 1) * CH)
                # kv-path chains first: they only need the (small) k latents
                for h in range(HPC):
                    ps = pp.tile([128, CH], F32, tag="ups")
                    for m in range(KVT):
                        nc.tensor.matmul(
                            ps[:],
                            wkn_s[:, m * HPC * NOPE + h * NOPE:
                                  m * HPC * NOPE + (h + 1) * NOPE],
                            lk[m],
                            start=(m == 0), stop=(m == KVT - 1))
                    nc.scalar.copy(kn_T[h][:, csl], ps[:])
                for st in range(CH // 128):
                    ps = pp.tile([128, CH], F32, tag="ups")
                    for m in range(KVT):
                        nc.tensor.matmul(
                            ps[:, :HPC * VH],
                            lk[m][:, st * 128:(st + 1) * 128],
                            wkv_s[:, m * HPC * VH:(m + 1) * HPC * VH],
                            start=(m == 0), stop=(m == KVT - 1))
                    gst = c * (CH // 128) + st
                    nc.scalar.copy(
                        v2[:, gst * HPC * VH:(gst + 1) * HPC * VH],
                        ps[:, :HPC * VH])
                for h in range(HPC):
                    ps = pp.tile([128, CH], F32, tag="ups")
                    for m in range(QLT):
                        nc.tensor.matmul(
                            ps[:],
                            wqn_s[:, m * HPC * NOPE + h * NOPE:
                                  m * HPC * NOPE + (h + 1) * NOPE],
                            lq[m],
                            start=(m == 0), stop=(m == QLT - 1))
                    nc.vector.tensor_copy(qn_T[h][:, csl], ps[:])
                ps = pp.tile([128, CH], F32, tag="ups")
                for m in range(QLT):
                    nc.tensor.matmul(ps[:],
                                     wqr_s[:, m * HPC * 64:(m + 1) * HPC * 64],
                                     lq[m],
                                     start=(m == 0), stop=(m == QLT - 1))
                nc.vector.tensor_copy(qr2_T[:, csl], ps[:])

            def attention(qc):
                """Causal attention for query chunk qc; returns ot tiles.

                Full key tiles first, then the 4 diagonal tiles restricted
                to their unmasked query columns. The exp sums accumulate in
                SBUF on GpSimd (even tiles) and DVE (odd tiles); one f32r
                ones-matmul per head turns the sum into the denominator.
                """
                qb = qc * QC
                tiles = [(kt, 0) for kt in range(4 * qc)]
                tiles += [(4 * qc + d, 128 * d) for d in range(4)]
                n = len(tiles)
                ot = []
                for h in range(HPC):
                    ps_o = pvp.tile([128, QC], F32, tag="po")
                    es = []
                    for p in range(2):
                        t = esp.tile([128, QC], F32, tag=f"es{p}")
                        (nc.gpsimd if p == 0 else nc.vector).memset(t[:], 0.0)
                        es.append(t)
                    prev = None

                    def pv(i, kt, off, et):
                        w = QC - off
                        nc.tensor.matmul(
                            ps_o[:, off:],
                            v2[:, kt * HPC * VH + h * VH:
                               kt * HPC * VH + (h + 1) * VH],
                            et[:, :w], start=(i == 0), stop=(i == n - 1))

                    for i, (kt, off) in enumerate(tiles):
                        w = QC - off
                        ps_s = pp.tile([128, QC], F32, tag="ups")
                        nc.tensor.matmul(ps_s[:, :w],
                                         kn_T[h][:, kt * 128:(kt + 1) * 128],
                                         qn_T[h][:, qb + off:qb + QC],
                                         start=True, stop=False)
                        nc.tensor.matmul(
                            ps_s[:, :w],
                            kr2_T[h * 64:(h + 1) * 64,
                                  kt * 128:(kt + 1) * 128],
                            qr2_T[h * 64:(h + 1) * 64, qb + off:qb + QC],
                            start=False, stop=True)
                        if kt >= 4 * qc:    # diagonal tile: staircase mask
                            nc.vector.tensor_add(
                                ps_s[:, :128], ps_s[:, :128], md_s[:])
                        et = ep.tile([128, QC], BF16, tag="et")
                        nc.scalar.activation(
                            et[:, :w], ps_s[:, :w],
                            mybir.ActivationFunctionType.Exp,
                            bias=zb[:], scale=1.0)
                        eng = nc.gpsimd if i % 2 == 0 else nc.vector
                        e = es[i % 2]
                        eng.tensor_add(e[:, off:], e[:, off:], et[:, :w])
                        if prev is not None:
                            pv(*prev)
                        prev = (i, kt, off, et)
                    pv(*prev)
                    est = tp.tile([128, QC], F32R, tag="est")
                    nc.vector.tensor_add(est[:], es[0][:], es[1][:])
                    ps_den = pdp.tile([1, QC], F32, tag="den")
                    nc.tensor.matmul(ps_den[:], ones, est[:],
                                     start=True, stop=True)
                    rd = tp.tile([1, QC], F32, tag="rd")
                    dencp = tp.tile([1, QC], F32, tag="dencp")
                    nc.vector.tensor_copy(dencp[:], ps_den[:])
                    nc.vector.reciprocal_approx_fast(rd[:], dencp[:])
                    rdb = tp.tile([128, QC], F32, tag="rdb")
                    nc.gpsimd.partition_broadcast(rdb[:], rd[:1])
                    o = otp.tile([128, QC], BF16, tag=f"ot{h}")
                    nc.vector.tensor_mul(o[:], ps_o[:], rdb[:])
                    ot.append(o)
                return ot

            def o_proj(qc, ot):
                for st in range(QC // 128):
                    foc = fop.tile([128, HID], BF16, tag="fo")
                    for nn in range(HID // 512):
                        ps_f = pop.tile([128, 512], F32, tag="pf")
                        for h in range(HPC):
                            nc.tensor.matmul(
                                ps_f[:],
                                ot[h][:, st * 128:(st + 1) * 128],
                                wo_s[:, h * HID + nn * 512:
                                     h * HID + (nn + 1) * 512],
                                start=(h == 0), stop=(h == HPC - 1))
                        if nn % 2 == 0:
                            nc.vector.tensor_copy(
                                foc[:, nn * 512:(nn + 1) * 512], ps_f[:])
                        else:
                            nc.scalar.copy(
                                foc[:, nn * 512:(nn + 1) * 512], ps_f[:])
                    for p in range(2):  # full 4KB rows
                        nc.sync.dma_start(
                            part[qc * QC + st * 128 + p * 64:
                                 qc * QC + st * 128 + (p + 1) * 64, :],
                            foc[p * 64:(p + 1) * 64, :])

            chunks = pend
            prev_ot = None
            for c in range(NCH):
                if c == 1:
                    chunks.append(load_chunk(3))
                lq, lk = chunks[c]
                up_proj(c, lq, lk)
                if prev_ot is not None:
                    o_proj(c - 1, prev_ot)
                prev_ot = attention(c)
            o_proj(NCH - 1, prev_ot)
    nc.compile()
    return nc


def _build_b_general():
    """Fallback launch B for arbitrary masks: full [S,S] mask, no tile
    skipping (bf16 activations)."""
    nc = bacc.Bacc("TRN2", target_bir_lowering=False, debug=False,
                   num_devices=NCORES)
    qlat = nc.dram_tensor("qlat", [QL, S], BF16, kind="ExternalInput").ap()
    kvlat = nc.dram_tensor("kvlat", [KVL, S], BF16,
                           kind="ExternalInput").ap()
    rp = nc.dram_tensor("rp", [ROPE, S], BF16, kind="ExternalInput").ap()
    maskT = nc.dram_tensor("maskT", [S, S], BF16,
                           kind="ExternalInput").ap()
    wqn = nc.dram_tensor("wqn", [128, QLT * HPC * NOPE], BF16,
                         kind="ExternalInput").ap()
    wqr = nc.dram_tensor("wqr", [128, QLT * HPC * 64], BF16,
                         kind="ExternalInput").ap()
    wkn = nc.dram_tensor("wkn", [128, KVT * HPC * NOPE], BF16,
                         kind="ExternalInput").ap()
    wkv = nc.dram_tensor("wkv", [128, KVT * HPC * VH], BF16,
                         kind="ExternalInput").ap()
    wo = nc.dram_tensor("wo", [HPC * VH, HID], BF16,
                        kind="ExternalInput").ap()
    part = nc.dram_tensor("part", [S, HID], BF16, kind="ExternalOutput").ap()

    CH = 512
    NCH = S // CH
    QC = CH

    with tile.TileContext(nc) as tc:
        with tc.tile_pool(name="w", bufs=1) as wp, \
             tc.tile_pool(name="act", bufs=1) as ap_, \
             tc.tile_pool(name="lq", bufs=2) as lqp, \
             tc.tile_pool(name="msk", bufs=24) as mp, \
             tc.tile_pool(name="tmp", bufs=2) as tp, \
             tc.tile_pool(name="et", bufs=3) as ep, \
             tc.tile_pool(name="out", bufs=5) as op, \
             tc.tile_pool(name="ps", bufs=2, space="PSUM") as pp, \
             tc.tile_pool(name="psden", bufs=2, space="PSUM") as pdp, \
             tc.tile_pool(name="pspv", bufs=2, space="PSUM") as pvp, \
             tc.tile_pool(name="pso", bufs=2, space="PSUM") as pop:
            ones_b = wp.tile([128, 1], BF16, tag="ones")
            nc.vector.memset(ones_b[:], 1.0)
            ones = ones_b[:]
            zb = wp.tile([128, 1], F32, tag="zb")
            nc.vector.memset(zb[:], 0.0)

            qn_T = [ap_.tile([128, S], BF16, tag=f"qnT{h}", name=f"qnT{h}")
                    for h in range(HPC)]
            qr2_T = ap_.tile([128, S], BF16, tag="qr2T")
            kn_T = [ap_.tile([128, S], BF16, tag=f"knT{h}", name=f"knT{h}")
                    for h in range(HPC)]
            v2 = ap_.tile([128, ST * HPC * VH], BF16, tag="v2")
            kr2_T = ap_.tile([128, S], BF16, tag="kr2T")

            def load_chunk(c):
                csl = slice(c * CH, (c + 1) * CH)
                lq = lqp.tile([128, QLT * CH], BF16, tag="lq", name="lq")
                for m in range(QLT):
                    nc.sync.dma_start(lq[:, m * CH:(m + 1) * CH],
                                      qlat[m * 128:(m + 1) * 128, csl])
                lk = lqp.tile([128, KVT * CH], BF16, tag="lk", name="lk")
                for m in range(KVT):
                    nc.sync.dma_start(lk[:, m * CH:(m + 1) * CH],
                                      kvlat[m * 128:(m + 1) * 128, csl])
                nc.sync.dma_start(kr2_T[0:64, csl], rp[:, csl])
                nc.sync.dma_start(kr2_T[64:128, csl], rp[:, csl])
                return lq, lk

            pend = load_chunk(0)
            wqn_s = wp.tile([128, QLT * HPC * NOPE], BF16, tag="wqn")
            for m in range(QLT):
                nc.sync.dma_start(
                    wqn_s[:, m * HPC * NOPE:(m + 1) * HPC * NOPE],
                    wqn[:, m * HPC * NOPE:(m + 1) * HPC * NOPE])
            wqr_s = wp.tile([128, QLT * HPC * 64], BF16, tag="wqr")
            nc.sync.dma_start(wqr_s[:], wqr[:, :])
            wkn_s = wp.tile([128, KVT * HPC * NOPE], BF16, tag="wkn")
            nc.sync.dma_start(wkn_s[:], wkn[:, :])
            wkv_s = wp.tile([128, KVT * HPC * VH], BF16, tag="wkv")
            nc.sync.dma_start(wkv_s[:], wkv[:, :])
            wo_s = wp.tile([128, HPC * HID], BF16, tag="wo")
            for h in range(HPC):
                nc.sync.dma_start(wo_s[:, h * HID:(h + 1) * HID],
                                  wo[h * 128:(h + 1) * 128, :])

            def up_proj(c, lq, lk):
                csl = slice(c * CH, (c + 1) * CH)
                for h in range(HPC):
                    ps = pp.tile([128, CH], F32, tag="ups")
                    for m in range(QLT):
                        nc.tensor.matmul(
                            ps[:],
                            wqn_s[:, m * HPC * NOPE + h * NOPE:
                                  m * HPC * NOPE + (h + 1) * NOPE],
                            lq[:, m * CH:(m + 1) * CH],
                            start=(m == 0), stop=(m == QLT - 1))
                    nc.vector.tensor_copy(qn_T[h][:, csl], ps[:])
                ps = pp.tile([128, CH], F32, tag="ups")
                for m in range(QLT):
                    nc.tensor.matmul(ps[:],
                                     wqr_s[:, m * HPC * 64:(m + 1) * HPC * 64],
                                     lq[:, m * CH:(m + 1) * CH],
                                     start=(m == 0), stop=(m == QLT - 1))
                nc.vector.tensor_copy(qr2_T[:, csl], ps[:])
                for h in range(HPC):
                    ps = pp.tile([128, CH], F32, tag="ups")
                    for m in range(KVT):
                        nc.tensor.matmul(
                            ps[:],
                            wkn_s[:, m * HPC * NOPE + h * NOPE:
                                  m * HPC * NOPE + (h + 1) * NOPE],
                            lk[:, m * CH:(m + 1) * CH],
                            start=(m == 0), stop=(m == KVT - 1))
                    nc.scalar.copy(kn_T[h][:, csl], ps[:])
                for st in range(CH // 128):
                    ps = pp.tile([128, HPC * VH], F32, tag="ups")
                    for m in range(KVT):
                        nc.tensor.matmul(
                            ps[:],
                            lk[:, m * CH + st * 128:m * CH + (st + 1) * 128],
                            wkv_s[:, m * HPC * VH:(m + 1) * HPC * VH],
                            start=(m == 0), stop=(m == KVT - 1))
                    gst = c * (CH // 128) + st
                    nc.scalar.copy(
                        v2[:, gst * HPC * VH:(gst + 1) * HPC * VH], ps[:])

            for c in range(NCH):
                lq, lk = pend
                if c + 1 < NCH:
                    pend = load_chunk(c + 1)
                up_proj(c, lq, lk)

            def attention(qc):
                qsl = slice(qc * QC, (qc + 1) * QC)
                mts = []
                for kt in range(ST):
                    mt = mp.tile([128, QC], BF16, tag="mask")
                    nc.sync.dma_start(mt[:],
                                      maskT[kt * 128:(kt + 1) * 128, qsl])
                    mts.append(mt)
                ot = []
                for h in range(HPC):
                    ps_den = pdp.tile([1, QC], F32, tag="den")
                    ps_o = pvp.tile([128, QC], F32, tag="po")
                    ets = {}
                    for kt in range(ST):
                        ps_s = pp.tile([128, QC], F32, tag="ups")
                        nc.tensor.matmul(ps_s[:],
                                         kn_T[h][:, kt * 128:(kt + 1) * 128],
                                         qn_T[h][:, qsl],
                                         start=True, stop=False)
                        nc.tensor.matmul(
                            ps_s[:],
                            kr2_T[h * 64:(h + 1) * 64,
                                  kt * 128:(kt + 1) * 128],
                            qr2_T[h * 64:(h + 1) * 64, qsl],
                            start=False, stop=True)
                        nc.vector.tensor_add(ps_s[:], ps_s[:], mts[kt][:])
                        et = ep.tile([128, QC], BF16, tag="et")
                        nc.scalar.activation(
                            et[:], ps_s[:], mybir.ActivationFunctionType.Exp,
                            bias=zb[:], scale=1.0)
                        ets[kt] = et
                        if kt > 0:
                            pkt = kt - 1
                            pet = ets.pop(pkt)
                            nc.tensor.matmul(
                                ps_o[:],
                                v2[:, pkt * HPC * VH + h * VH:
                                   pkt * HPC * VH + (h + 1) * VH],
                                pet[:], start=(pkt == 0), stop=False)
                            nc.tensor.matmul(ps_den[:], ones, pet[:],
                                             start=(pkt == 0), stop=False)
                    pkt = ST - 1
                    pet = ets.pop(pkt)
                    nc.tensor.matmul(
                        ps_o[:],
                        v2[:, pkt * HPC * VH + h * VH:
                           pkt * HPC * VH + (h + 1) * VH],
                        pet[:], start=(pkt == 0), stop=True)
                    nc.tensor.matmul(ps_den[:], ones, pet[:],
                                     start=(pkt == 0), stop=True)
                    rd = tp.tile([1, QC], F32, tag="rd")
                    dencp = tp.tile([1, QC], F32, tag="dencp")
                    nc.vector.tensor_copy(dencp[:], ps_den[:])
                    nc.vector.reciprocal_approx_fast(rd[:], dencp[:])
                    rdb = tp.tile([128, QC], F32, tag="rdb")
                    nc.gpsimd.partition_broadcast(rdb[:], rd[:1])
                    o = op.tile([128, QC], BF16, tag=f"ot{h}")
                    nc.vector.tensor_mul(o[:], ps_o[:], rdb[:])
                    ot.append(o)
                return ot

            def o_proj(qc, ot):
                for st in range(QC // 128):
                    for nn in range(HID // 512):
                        ps_f = pop.tile([128, 512], F32, tag="pf")
                        for h in range(HPC):
                            nc.tensor.matmul(
                                ps_f[:],
                                ot[h][:, st * 128:(st + 1) * 128],
                                wo_s[:, h * HID + nn * 512:
                                     h * HID + (nn + 1) * 512],
                                start=(h == 0), stop=(h == HPC - 1))
                        fo = op.tile([128, 512], BF16, tag="fo")
                        nc.scalar.copy(fo[:], ps_f[:])
                        nc.sync.dma_start(
                            part[qc * QC + st * 128:qc * QC + (st + 1) * 128,
                                 nn * 512:(nn + 1) * 512], fo[:])

            prev_ot = None
            for qc in range(NCH):
                if prev_ot is not None:
                    o_proj(qc - 1, prev_ot)
                prev_ot = attention(qc)
            o_proj(NCH - 1, prev_ot)
    nc.compile()
    return nc


def _check_causal128(maskT):
    """True iff maskT ([k, q], f32) is block-causal at 128x128 tile
    granularity with one shared diagonal pattern; returns (ok, P[128,128])."""
    P = None
    for qt in range(ST):
        for kt in range(ST):
            blk = maskT[kt * 128:(kt + 1) * 128, qt * 128:(qt + 1) * 128]
            if kt < qt:
                if not np.all(blk == 0.0):
                    return False, None
            elif kt > qt:
                if not np.all(blk <= -1e8):
                    return False, None
            elif P is None:
                P = blk
            elif not np.array_equal(P, blk):
                return False, None
    return True, P


def _get(name):
    if name not in _CACHE:
        _CACHE[name] = {"a": _build_a, "bc": _build_b_causal,
                        "bg": _build_b_general}[name]()
    return _CACHE[name]


def _prep(hidden_states, attention_mask, Wqa, gqa, Wqb, Wkva, gkva, Wkvb, Wo):
    import ml_dtypes
    f = np.float32
    bf = ml_dtypes.bfloat16
    hid_T = np.ascontiguousarray(hidden_states[0].T).astype(bf)
    mask_T = np.ascontiguousarray(
        np.asarray(attention_mask[0, 0], f).T)
    ok, mstair = _check_causal128(mask_T)
    Wqb_g = (np.asarray(gqa, f)[:, None] * np.asarray(Wqb, f)).astype(f)
    Wkvb_g = (np.asarray(gkva, f)[:, None] * np.asarray(Wkvb, f)).astype(f)
    # launch-A weight layouts: hid-partition-major, j(-contraction)-sliced
    wqa_np = np.asarray(Wqa, f)
    wkva_np = np.asarray(Wkva, f)
    wq_b = np.ascontiguousarray(
        wqa_np.reshape(HT, 128, QFC, 512).transpose(1, 2, 0, 3)
        .reshape(128, QFC * HT * 512)).astype(bf)
    wkv_b = np.ascontiguousarray(
        wkva_np[:, :KVL].reshape(HT, 128, KVL).transpose(1, 0, 2)
        .reshape(128, HT * KVL)).astype(bf)
    wrp_b = np.ascontiguousarray(
        wkva_np[:, KVL:].reshape(HT, 128, ROPE).transpose(1, 0, 2)
        .reshape(128, HT * ROPE)).astype(bf)
    ins_a, ins_b = [], []
    for c in range(NCORES):
        hsl_c = np.ascontiguousarray(
            hid_T[:, c * SL:(c + 1) * SL].reshape(HT, 128, SL)
            .transpose(1, 0, 2).reshape(128, HT * SL))
        ins_a.append({
            "hsl": hsl_c,
            "wq": wq_b,
            "wkv": wkv_b,
            "wrp": wrp_b,
        })
        heads = [HPC * c + h for h in range(HPC)]
        wqn = np.concatenate([Wqb_g[:, h * 192:h * 192 + NOPE] for h in heads],
                             axis=1)
        wqr = np.concatenate([Wqb_g[:, h * 192 + NOPE:(h + 1) * 192]
                              for h in heads], axis=1)
        wkn = np.concatenate([Wkvb_g[:, h * 256:h * 256 + NOPE]
                              for h in heads], axis=1)
        wkv = np.concatenate([Wkvb_g[:, h * 256 + NOPE:(h + 1) * 256]
                              for h in heads], axis=1)
        wo = np.concatenate([np.asarray(Wo, f)[h * VH:(h + 1) * VH, :]
                             for h in heads], axis=0)
        mask_in = ({"mstair": mstair.astype(bf)} if ok
                   else {"maskT": mask_T.astype(bf)})

        def perm(w, nt):
            # [nt*128, F] -> [128, nt*F] tile-major contiguous
            return np.ascontiguousarray(
                w.reshape(nt, 128, w.shape[1]).transpose(1, 0, 2)
                .reshape(128, nt * w.shape[1])).astype(bf)

        ins_b.append({
            **mask_in,
            "wqn": perm(wqn, QLT),
            "wqr": perm(wqr, QLT),
            "wkn": perm(wkn, KVT),
            "wkv": perm(wkv, KVT),
            "wo": np.ascontiguousarray(wo).astype(bf),
        })
    return ins_a, ins_b, ("bc" if ok else "bg")


def _run(ins_a, ins_b, bname="bc", trace=False):
    core_ids = list(range(NCORES))
    res_a = run_bass_kernel_spmd(_get("a"), ins_a, core_ids, trace=trace)
    qlat = np.ascontiguousarray(np.concatenate(
        [res_a.results[c]["qtok"] for c in range(NCORES)], axis=0).T)
    kvr = np.concatenate([res_a.results[c]["kvr"] for c in range(NCORES)],
                         axis=0)
    kvlat = np.ascontiguousarray(kvr[:, :KVL].T)
    rplat = np.ascontiguousarray(kvr[:, KVL:].T)
    for m in ins_b:
        m["qlat"] = qlat
        m["kvlat"] = kvlat
        m["rp"] = rplat
    res_b = run_bass_kernel_spmd(_get(bname), ins_b, core_ids, trace=trace)
    out = res_b.results[0]["part"].astype(np.float32)
    for c in range(1, NCORES):
        out = out + res_b.results[c]["part"].astype(np.float32)
    return out[None], res_a, res_b


def kernel(hidden_states, attention_mask, Wqa, gqa, Wqb, Wkva, gkva, Wkvb, Wo):
    ins_a, ins_b, bname = _prep(hidden_states, attention_mask, Wqa, gqa, Wqb,
                                Wkva, gkva, Wkvb, Wo)
    out, _, _ = _run(ins_a, ins_b, bname)
    return out
